# revision 1
# baseline (speedup 1.0000x reference)
"""Talking-heads attention Trainium2 kernel (Bass/Tile), 8-core data-parallel.

Problem: nn_Attention_talking_head — B=64, N=245, C=768, H=12, D=64,
RPE table (12, 1698) indexed by rel_idx (245, 245), talking-heads mixing
(12x12) before and after softmax, in/out projections.

Sharding: batch 64 -> 8 cores x 8 batches. Weights replicated. No collectives.

Per-core pipeline (all compute on device):
  phase 0: transpose weights via PE; premix RPE table with W_l (one matmul);
           gather premixed bias via gpsimd ap_gather (custom ucode op);
           repack bias into the packed (head, n-slot) layout via 12 SBUF DMAs.
  per b:   x -> xT (PE transpose); QKV GEMM (fp32r); per-head QK^T into a wide
           S^T [m, (h,n)] buffer; strided-column PE transposes into packed
           tiles [(h, nb), m] (nb = gather group 0..7, n = 31*nb + j);
           pre-softmax head-mix = one 96x96 block-diagonal matmul per j;
           fused bias-add + softmax (reduce_max -> Exp w/ accum sum -> scale);
           post-softmax mix FUSED with the transpose back (lhsT = P trick)
           giving A'^T [m, (h,n)]; AV per head; +b_w * colsum(v); out proj.

b_l is mathematically a no-op (constant per softmax row) and is skipped.
"""
import numpy as np
from contextlib import ExitStack

import concourse.bass as bass
import concourse.tile as tile
from concourse import bacc, mybir, library_config
from concourse.bass_utils import run_bass_kernel_spmd
from concourse.masks import make_identity

F32 = mybir.dt.float32
F32R = mybir.dt.float32r
BF16 = mybir.dt.bfloat16
I32 = mybir.dt.int32
I16 = mybir.dt.int16
AX = mybir.AxisListType.X
EXP = mybir.ActivationFunctionType.Exp
ADD = mybir.AluOpType.add
MULT = mybir.AluOpType.mult

NCORES = 8
B, N, C, H, D = 64, 245, 768, 12, 64
BLOC = B // NCORES          # 8 batches per core
E = 3 * C                   # 2304
NBKT = 1698
SCALE = D ** -0.5
NPAD = 256                  # padded n stride (free >= 256 keeps fp32r at 1 cyc/row)
NGRP = 8                    # gather groups == packed nb slots
NJ = 31                     # packed tiles per batch; n = 31*nb + j, j in [0, NJ)
NIDX = 7600                 # gather stream length per group (31*245 real + 5 pad)
CC = C // 128               # 6 contraction chunks
MCS = [(0, 128), (128, 117)]  # (m offset, size) chunks of 245


def _emit(ctx: ExitStack, tc, io):
    nc = tc.nc
    x_d, wqkv_d, wproj_d, bproj_d, wl_d, ww_d, bw_d, rpe_d, rel_d, out_d = io

    const = ctx.enter_context(tc.tile_pool(name="const", bufs=1))
    ctx0 = ctx.enter_context(ExitStack())
    tmp = ctx0.enter_context(tc.tile_pool(name="tmp", bufs=1))
    ps_big = ctx.enter_context(tc.tile_pool(name="ps_big", bufs=2, space="PSUM"))
    ps_mid = ctx.enter_context(tc.tile_pool(name="ps_mid", bufs=2, space="PSUM"))
    ps_mix = ctx.enter_context(tc.tile_pool(name="ps_mix", bufs=2, space="PSUM"))
    ps_sml = ctx.enter_context(tc.tile_pool(name="ps_sml", bufs=2, space="PSUM"))

    ident = const.tile([128, 128], F32)
    make_identity(nc, ident[:])

    # ---- weight transposes (PE) ----
    wqkvT = const.tile([128, CC, E], F32R)   # [c-part, c-chunk, e]
    for ec in range(E // 128):
        wt = tmp.tile([128, C], F32, tag="wload")
        nc.sync.dma_start(out=wt[:], in_=wqkv_d[ec * 128:(ec + 1) * 128, :])
        for cc in range(CC):
            pst = ps_big.tile([128, 128], F32, tag="big")
            nc.tensor.transpose(out=pst[:], in_=wt[:, cc * 128:(cc + 1) * 128],
                                identity=ident[:])
            nc.scalar.copy(out=wqkvT[:, cc, ec * 128:(ec + 1) * 128], in_=pst[:])

    wprojT = const.tile([128, CC, C], F32R)
    for ec in range(CC):
        wt = tmp.tile([128, C], F32, tag="wload")
        nc.sync.dma_start(out=wt[:], in_=wproj_d[ec * 128:(ec + 1) * 128, :])
        for cc in range(CC):
            pst = ps_big.tile([128, 128], F32, tag="big")
            nc.tensor.transpose(out=pst[:], in_=wt[:, cc * 128:(cc + 1) * 128],
                                identity=ident[:])
            nc.scalar.copy(out=wprojT[:, cc, ec * 128:(ec + 1) * 128], in_=pst[:])

    # ---- w_l / w_w transposes; block-diagonal mixers ----
    wl_sb = tmp.tile([12, 12], F32, tag="wsml")
    nc.sync.dma_start(out=wl_sb[:], in_=wl_d[:, :])
    ps12 = ps_sml.tile([12, 12], F32, tag="sml")
    nc.tensor.transpose(out=ps12[:], in_=wl_sb[:], identity=ident[:12, :12])
    wlT_plain = const.tile([12, 12], F32)          # w_l^T (for RPE premix)
    nc.scalar.copy(out=wlT_plain[:], in_=ps12[:])
    wlT_scaled = tmp.tile([12, 12], F32, tag="wsml2")
    nc.scalar.mul(out=wlT_scaled[:], in_=ps12[:], mul=SCALE)

    ww_sb = tmp.tile([12, 12], F32, tag="wsml")
    nc.sync.dma_start(out=ww_sb[:], in_=ww_d[:, :])
    ps12b = ps_sml.tile([12, 12], F32, tag="sml")
    nc.tensor.transpose(out=ps12b[:], in_=ww_sb[:], identity=ident[:12, :12])
    wwT = tmp.tile([12, 12], F32, tag="wsml2")
    nc.scalar.copy(out=wwT[:], in_=ps12b[:])

    # nb-major packing: row p = nb*12 + h -> contiguous 12x12 diagonal blocks.
    # Engine writes must start at 32-aligned partitions, so assemble in f32
    # scratch via DMA block copies, then round to f32r with one aligned copy.
    bd1_f32 = tmp.tile([H * NGRP, H * NGRP], F32, tag="bd1f")
    nc.vector.memset(bd1_f32[:], 0.0)
    bd2_f32 = tmp.tile([H * NGRP, H * NGRP], F32, tag="bd2f")
    nc.vector.memset(bd2_f32[:], 0.0)
    for nb in range(NGRP):
        s = nb * H
        nc.gpsimd.dma_start(out=bd1_f32[s:s + H, s:s + H], in_=wlT_scaled[:])
        nc.gpsimd.dma_start(out=bd2_f32[s:s + H, s:s + H], in_=wwT[:])
    bd1 = const.tile([H * NGRP, H * NGRP], F32R)   # [(nb,h), (nb,g)] = SCALE*w_l[g,h]
    nc.scalar.copy(out=bd1[:], in_=bd1_f32[:])
    bd2 = const.tile([H * NGRP, H * NGRP], F32R)   # [(nb,g), (nb,h)] = w_w[h,g]
    nc.scalar.copy(out=bd2[:], in_=bd2_f32[:])

    # ---- premixed RPE table: mixed_rpe[g, k] = sum_h w_l[g,h] * rpe[h, k] ----
    rpe_sb = tmp.tile([12, NBKT], F32, tag="rpe")
    nc.sync.dma_start(out=rpe_sb[:], in_=rpe_d[:, :])
    mixed_rpe = tmp.tile([12, NBKT], F32, tag="rpemix")
    for o in range(0, NBKT, 512):
        w = min(512, NBKT - o)
        psr = ps_sml.tile([12, 512], F32, tag="sml")
        nc.tensor.matmul(out=psr[:, :w], lhsT=wlT_plain[:], rhs=rpe_sb[:, o:o + w],
                         start=True, stop=True)
        nc.scalar.copy(out=mixed_rpe[:, o:o + w], in_=psr[:, :w])

    # replicate across the 8 gather groups: table_rep[16*grp + c] = mixed_rpe[c]
    table_rep = tmp.tile([128, NBKT], F32, tag="trep")
    nc.vector.memset(table_rep[:], 0.0)
    for c in range(12):
        for grp in range(NGRP):
            p = grp * 16 + c
            nc.sync.dma_start(out=table_rep[p:p + 1, :], in_=mixed_rpe[c:c + 1, :])

    # ---- gather indices (wrapped int16 streams per 16-partition group) ----
    rel_flat = rel_d.rearrange("n m -> (n m)")
    idx32 = tmp.tile([128, NIDX // 16], I32, tag="idx32")
    nc.vector.memset(idx32[:], 0)
    for grp in range(NGRP):
        base = grp * NJ * N
        if grp < 7:
            nc.sync.dma_start(
                out=idx32[grp * 16:(grp + 1) * 16, :],
                in_=rel_flat[base:base + NIDX].rearrange("(s p) -> p s", p=16))
        else:
            # group 7 has 28 real n rows (6860 idxs): 16x428 full + 12 tail
            nc.sync.dma_start(
                out=idx32[grp * 16:(grp + 1) * 16, :428],
                in_=rel_flat[base:base + 6848].rearrange("(s p) -> p s", p=16))
            nc.sync.dma_start(
                out=idx32[grp * 16:grp * 16 + 12, 428:429],
                in_=rel_flat[base + 6848:base + 6860].rearrange("(s p) -> p s", p=12))
    idx16 = tmp.tile([128, NIDX // 16], I16, tag="idx16")
    nc.vector.tensor_copy(out=idx16[:], in_=idx32[:])

    # ---- gather premixed bias, then repack to [(h, nb), j*245 + m] ----
    nc.gpsimd.load_library(library_config.ap_gather)
    bias_g = tmp.tile([128, NIDX], F32, tag="biasg")
    nc.gpsimd.ap_gather(
        out_ap=bias_g[:], in_ap=table_rep[:].unsqueeze(2), idxs_ap=idx16[:],
        channels=128, num_elems=NBKT, d=1, num_idxs=NIDX)
    nc.gpsimd.load_library(library_config.standard)

    packed_bias = const.tile([H * NGRP, NJ * N], BF16)
    for h in range(12):
        for grp in range(NGRP):
            nc.gpsimd.dma_start(out=packed_bias[grp * H + h:grp * H + h + 1, :],
                                in_=bias_g[grp * 16 + h:grp * 16 + h + 1, :NJ * N])

    # ---- small constants ----
    bw_exp = const.tile([128, CC, 1], F32)   # b_w[(t*128+p)//64]
    for t in range(CC):
        for half in range(2):
            h_idx = 2 * t + half
            nc.gpsimd.dma_start(
                out=bw_exp[half * 64:(half + 1) * 64, t, :],
                in_=bw_d[h_idx:h_idx + 1].unsqueeze(0).to_broadcast([64, 1]))
    bproj_sb = const.tile([128, C], F32)
    nc.gpsimd.dma_start(out=bproj_sb[:], in_=bproj_d[:].unsqueeze(0).to_broadcast([128, C]))
    ones = const.tile([128, 1], F32)
    nc.vector.memset(ones[:], 1.0)
    zeros_c = const.tile([128, 1], F32)
    nc.vector.memset(zeros_c[:], 0.0)

    ctx0.close()

    # ---- per-batch streaming pools ----
    xb_p = ctx.enter_context(tc.tile_pool(name="xb", bufs=1))
    xT_p = ctx.enter_context(tc.tile_pool(name="xT", bufs=1))
    qT_p = ctx.enter_context(tc.tile_pool(name="qT", bufs=1))
    kT_p = ctx.enter_context(tc.tile_pool(name="kT", bufs=1))
    v_p = ctx.enter_context(tc.tile_pool(name="v", bufs=2))
    swt_p = ctx.enter_context(tc.tile_pool(name="swt", bufs=1))
    pk_p = ctx.enter_context(tc.tile_pool(name="pk", bufs=2))
    sm_p = ctx.enter_context(tc.tile_pool(name="sm", bufs=2))
    p_p = ctx.enter_context(tc.tile_pool(name="p", bufs=2))
    at_p = ctx.enter_context(tc.tile_pool(name="at", bufs=1))
    oT_p = ctx.enter_context(tc.tile_pool(name="oT", bufs=1))
    y_p = ctx.enter_context(tc.tile_pool(name="y", bufs=2))
    st_p = ctx.enter_context(tc.tile_pool(name="st", bufs=4))

    for b in range(BLOC):
        # ---- load x_b and transpose to xT [c, n] (fp32r, n padded to 256) ----
        xb = xb_p.tile([128, 2, C], F32)
        for mc, (mo, msz) in enumerate(MCS):
            nc.sync.dma_start(out=xb[:msz, mc, :], in_=x_d[b, mo:mo + msz, :])
        xT = xT_p.tile([128, CC, NPAD], F32R)
        nc.scalar.copy(out=xT[:, :, N:],
                       in_=zeros_c[:, 0:1].to_broadcast([128, CC, NPAD - N]))
        for mc, (mo, msz) in enumerate(MCS):
            for cc in range(CC):
                pst = ps_big.tile([128, 128], F32, tag="big")
                nc.tensor.transpose(out=pst[:, :msz], in_=xb[:msz, mc, cc * 128:(cc + 1) * 128],
                                    identity=ident[:msz, :msz])
                nc.scalar.copy(out=xT[:, cc, mo:mo + msz], in_=pst[:, :msz])

        # ---- QKV ----
        qT = qT_p.tile([128, CC, NPAD], F32R)     # [ (h,d) rows, n ] scaled later via bd1
        kT = kT_p.tile([128, CC, N], F32R)
        for ec in range(12):
            psq = ps_big.tile([128, NPAD], F32, tag="big")
            for cc in range(CC):
                nc.tensor.matmul(out=psq[:], lhsT=wqkvT[:, cc, ec * 128:(ec + 1) * 128],
                                 rhs=xT[:, cc, :], start=(cc == 0), stop=(cc == CC - 1))
            if ec < 6:
                nc.scalar.copy(out=qT[:, ec, :], in_=psq[:])
            else:
                nc.scalar.copy(out=kT[:, ec - 6, :], in_=psq[:, :N])
        v_sb = v_p.tile([128, 2, C], F32R)        # [m, (h,d)]
        for mc, (mo, msz) in enumerate(MCS):
            for vc in range(2):
                psv = ps_mid.tile([128, 384], F32, tag="mid")
                for cc in range(CC):
                    nc.tensor.matmul(
                        out=psv[:msz], lhsT=xT[:, cc, mo:mo + msz],
                        rhs=wqkvT[:, cc, 2 * C + vc * 384:2 * C + (vc + 1) * 384],
                        start=(cc == 0), stop=(cc == CC - 1))
                nc.scalar.copy(out=v_sb[:msz, mc, vc * 384:(vc + 1) * 384], in_=psv[:msz])

        # ---- b_w * colsum(v) ----
        bwv = st_p.tile([128, CC, 1], F32, tag="bwv")
        for t in range(CC):
            psvs = ps_sml.tile([128, 1], F32, tag="sml")
            for mc, (mo, msz) in enumerate(MCS):
                nc.tensor.matmul(out=psvs[:], lhsT=v_sb[:msz, mc, t * 128:(t + 1) * 128].bitcast(F32),
                                 rhs=ones[:msz, :].bitcast(F32),
                                 start=(mc == 0), stop=(mc == 1))
            nc.vector.tensor_tensor(out=bwv[:, t, :], in0=psvs[:], in1=bw_exp[:, t, :], op=MULT)

        # ---- QK^T, evicted into packed column order [m, (j, nb, h)] ----
        swt = swt_p.tile([128, 2, NJ * H * NGRP], F32)
        for mc, (mo, msz) in enumerate(MCS):
            for h in range(12):
                pss = ps_big.tile([128, NPAD], F32, tag="big")
                nc.tensor.matmul(
                    out=pss[:msz],
                    lhsT=kT[(h % 2) * 64:(h % 2) * 64 + 64, h // 2, mo:mo + msz],
                    rhs=qT[(h % 2) * 64:(h % 2) * 64 + 64, h // 2, :],
                    start=True, stop=True)
                nc.scalar.copy(
                    out=swt[:msz, mc, :].rearrange(
                        "p (j nb x) -> p j nb x", j=NJ, nb=NGRP)[:, :, :, h],
                    in_=pss[:msz, :NJ * NGRP].rearrange("p (nb j) -> p j nb", j=NJ))

        # ---- per-j packed attention ----
        atw = at_p.tile([128, 2, H, NPAD], F32R)   # A'^T wide
        for j in range(NJ):
            # T1: packed S [(h, nb), m]
            pk = pk_p.tile([H * NGRP, NPAD], F32R, tag="pk")
            for mc, (mo, msz) in enumerate(MCS):
                pspk = ps_sml.tile([H * NGRP, 128], F32, tag="sml")
                sel = swt[:msz, mc, j * 96:(j + 1) * 96]
                nc.tensor.transpose(out=pspk[:, :msz], in_=sel, identity=ident[:msz, :msz])
                if mc == 0:
                    nc.scalar.copy(out=pk[:, mo:mo + msz], in_=pspk[:, :msz])
                else:
                    nc.vector.tensor_copy(out=pk[:, mo:mo + msz], in_=pspk[:, :msz])
            # premix (block-diag) + bias add
            psm = ps_mix.tile([H * NGRP, NPAD], F32, tag="mix")
            nc.tensor.matmul(out=psm[:], lhsT=bd1[:], rhs=pk[:], start=True, stop=True)
            sm = sm_p.tile([H * NGRP, N], F32, tag="sm")
            nc.vector.tensor_tensor(out=sm[:], in0=psm[:, :N],
                                    in1=packed_bias[:, j * N:(j + 1) * N], op=ADD)
            # softmax over m
            negmax = st_p.tile([H * NGRP, 1], F32, tag="nm")
            nc.vector.reduce_max(out=negmax[:], in_=sm[:], axis=AX, negate=True)
            et = sm_p.tile([H * NGRP, N], F32, tag="et")
            ssum = st_p.tile([H * NGRP, 1], F32, tag="ss")
            nc.scalar.activation(out=et[:], in_=sm[:], func=EXP,
                                 bias=negmax[:], scale=1.0, accum_out=ssum[:])
            rec = st_p.tile([H * NGRP, 1], F32, tag="rc")
            nc.vector.reciprocal(out=rec[:], in_=ssum[:])
            pj = p_p.tile([H * NGRP, NPAD], F32R, tag="pj")
            nc.vector.tensor_scalar_mul(pj[:, :N], et[:], rec[:])
            # post-softmax mix fused with transpose back: A'^T = P^T-mixed
            for mc, (mo, msz) in enumerate(MCS):
                psat = ps_sml.tile([128, H * NGRP], F32, tag="sml")
                nc.tensor.matmul(out=psat[:msz], lhsT=pj[:, mo:mo + msz], rhs=bd2[:],
                                 start=True, stop=True)
                nc.vector.tensor_copy(out=atw[:msz, mc, :, j:j + 218:NJ].transpose([0, 2, 1]),
                                      in_=psat[:msz].rearrange("m (n h) -> m n h", h=H))

        # ---- AV (+ b_w colsum term) -> outT [(h,d), n] ----
        outT = oT_p.tile([128, CC, N], F32R)
        for h in range(12):
            psav = ps_mix.tile([64, NPAD], F32, tag="mix")
            for mc, (mo, msz) in enumerate(MCS):
                nc.tensor.matmul(out=psav[:], lhsT=v_sb[:msz, mc, h * 64:(h + 1) * 64],
                                 rhs=atw[:msz, mc, h, :], start=(mc == 0), stop=(mc == 1))
            nc.scalar.activation(
                out=outT[(h % 2) * 64:(h % 2) * 64 + 64, h // 2, :],
                in_=psav[:, :N], func=mybir.ActivationFunctionType.Identity,
                bias=bwv[(h % 2) * 64:(h % 2) * 64 + 64, h // 2, :], scale=1.0)

        # ---- projection + b_proj -> y -> DRAM ----
        for mc, (mo, msz) in enumerate(MCS):
            y = y_p.tile([128, C], F32)
            for half in range(2):
                psy = ps_mid.tile([128, 384], F32, tag="mid")
                for cc in range(CC):
                    nc.tensor.matmul(
                        out=psy[:msz], lhsT=outT[:, cc, mo:mo + msz],
                        rhs=wprojT[:, cc, half * 384:(half + 1) * 384],
                        start=(cc == 0), stop=(cc == CC - 1))
                nc.vector.tensor_tensor(out=y[:msz, half * 384:(half + 1) * 384],
                                        in0=psy[:msz],
                                        in1=bproj_sb[:msz, half * 384:(half + 1) * 384],
                                        op=ADD)
            nc.sync.dma_start(out=out_d[b, mo:mo + msz, :], in_=y[:msz, :])


_CACHE = {}


def _build():
    if "nc" in _CACHE:
        return _CACHE["nc"]
    nc = bacc.Bacc("TRN2", target_bir_lowering=False, debug=False, num_devices=NCORES)
    io = (
        nc.dram_tensor("x", [BLOC, N, C], F32, kind="ExternalInput").ap(),
        nc.dram_tensor("w_qkv", [E, C], F32, kind="ExternalInput").ap(),
        nc.dram_tensor("w_proj", [C, C], F32, kind="ExternalInput").ap(),
        nc.dram_tensor("b_proj", [C], F32, kind="ExternalInput").ap(),
        nc.dram_tensor("w_l", [H, H], F32, kind="ExternalInput").ap(),
        nc.dram_tensor("w_w", [H, H], F32, kind="ExternalInput").ap(),
        nc.dram_tensor("b_w", [H], F32, kind="ExternalInput").ap(),
        nc.dram_tensor("rpe_table", [H, NBKT], F32, kind="ExternalInput").ap(),
        nc.dram_tensor("rel_idx", [N, N], I32, kind="ExternalInput").ap(),
        nc.dram_tensor("out", [BLOC, N, C], F32, kind="ExternalOutput").ap(),
    )
    with tile.TileContext(nc) as tc, ExitStack() as ctx:
        _emit(ctx, tc, io)
    nc.compile()
    _CACHE["nc"] = nc
    return nc


def kernel(x, w_qkv, w_proj, b_proj, w_l, b_l, w_w, b_w, rpe_table, rel_idx,
           _trace=False):
    nc = _build()
    shared = {
        "w_qkv": np.ascontiguousarray(w_qkv, np.float32),
        "w_proj": np.ascontiguousarray(w_proj, np.float32),
        "b_proj": np.ascontiguousarray(b_proj, np.float32),
        "w_l": np.ascontiguousarray(w_l, np.float32),
        "w_w": np.ascontiguousarray(w_w, np.float32),
        "b_w": np.ascontiguousarray(b_w, np.float32),
        "rpe_table": np.ascontiguousarray(rpe_table, np.float32),
        "rel_idx": np.ascontiguousarray(rel_idx, np.int32),
    }
    x = np.ascontiguousarray(x, np.float32)
    in_maps = [dict(shared, x=x[i * BLOC:(i + 1) * BLOC]) for i in range(NCORES)]
    res = run_bass_kernel_spmd(nc, in_maps, core_ids=list(range(NCORES)),
                               trace=_trace)
    out = np.concatenate([res.results[i]["out"] for i in range(NCORES)], axis=0)
    if _trace:
        kernel.last_result = res
    return out



# revision 10
# speedup vs baseline: 1.1464x; 1.1464x over previous
"""Talking-heads attention Trainium2 kernel (Bass/Tile), 8-core data-parallel.

Problem: nn_Attention_talking_head — B=64, N=245, C=768, H=12, D=64,
RPE table (12, 1698) indexed by rel_idx (245, 245), talking-heads mixing
(12x12) before and after softmax, in/out projections.

Sharding: batch 64 -> 8 cores x 8 batches. Weights replicated. No collectives.

v2 design notes (vs v1):
  - bf16 attention pipeline (qT/kT/swt/pk/et/atw/v_sb + mixers); f32r GEMM
    anchors for QKV-from-x and the output projection.
  - max-subtraction dropped (|logit| <= ~2.5 for this problem's scales).
  - RPE bias add folded into the premix as a second accumulating matmul
    (identity lhsT), so softmax reads PSUM directly.
  - softmax normalization folded into the postmix by row-scaling the small
    w_w block-diagonal mixer with 1/rowsum (Pool engine).
  - per-tag PSUM pools (big/t1/mix/at = 2+2+2+2 banks) so consecutive j
    iterations pipeline instead of serializing on bank reuse.
  - paired evictions: QKV 2 e-chunks/bank, QK^T 2 heads/bank, T1 2 j/bank,
    postmix 2 j/bank -> fewer fixed-cost DVE/Act instructions.
  - atw uses j-major column order (col = j*8+nb); the AV eviction un-permutes
    back to n order via strided views.

b_l is mathematically a no-op (constant per softmax row) and is skipped.
"""
import os
import numpy as np
from contextlib import ExitStack

import concourse.bass as bass
import concourse.tile as tile
from concourse import bacc, mybir, library_config
from concourse.bass_utils import run_bass_kernel_spmd
from concourse.masks import make_identity

F32 = mybir.dt.float32
F32R = mybir.dt.float32r
BF16 = mybir.dt.bfloat16
I32 = mybir.dt.int32
I16 = mybir.dt.int16
AX = mybir.AxisListType.X
EXP = mybir.ActivationFunctionType.Exp
IDENT = mybir.ActivationFunctionType.Identity
ADD = mybir.AluOpType.add
MULT = mybir.AluOpType.mult

NCORES = 8
B, N, C, H, D = 64, 245, 768, 12, 64
BLOC = B // NCORES          # 8 batches per core
E = 3 * C                   # 2304
NBKT = 1698
SCALE = D ** -0.5
NPAD = 256
NGRP = 8                    # gather groups == packed nb slots
NJ = 31                     # packed tiles per batch; n = 31*nb + j, j in [0, NJ)
NJP = NJ * NGRP             # 248 packed col slots
NIDX = 7600                 # gather stream length per group (31*245 real + 5 pad)
CC = C // 128               # 6 contraction chunks
MCS = [(0, 128), (128, 117)]
P96 = H * NGRP              # 96 packed rows


def _emit(ctx: ExitStack, tc, io):
    nc = tc.nc
    x_d, wqkv_d, wproj_d, bproj_d, wl_d, ww_d, bw_d, rpe_d, rel_d, out_d = io

    const = ctx.enter_context(tc.tile_pool(name="const", bufs=1))
    ctx0 = ctx.enter_context(ExitStack())
    tmp = ctx0.enter_context(tc.tile_pool(name="tmp", bufs=1))
    ps_big = ctx.enter_context(tc.tile_pool(name="ps_big", bufs=2, space="PSUM"))
    ps_t1 = ctx.enter_context(tc.tile_pool(name="ps_t1", bufs=2, space="PSUM"))
    ps_mix = ctx.enter_context(tc.tile_pool(name="ps_mix", bufs=2, space="PSUM"))
    ps_at = ctx.enter_context(tc.tile_pool(name="ps_at", bufs=2, space="PSUM"))

    ident = const.tile([128, 128], F32)
    make_identity(nc, ident[:])
    ident_bf = const.tile([128, 128], BF16)
    nc.vector.tensor_copy(out=ident_bf[:], in_=ident[:])

    # ---- weight transposes (PE), paired evictions ----
    wqkvT = const.tile([128, CC, E], F32R)   # [c-part, c-chunk, e]
    for ec in range(E // 128):
        wt = tmp.tile([128, C], F32, tag="wload")
        nc.sync.dma_start(out=wt[:], in_=wqkv_d[ec * 128:(ec + 1) * 128, :])
        for cp in range(CC // 2):
            pst = ps_big.tile([128, 2, 128], F32, tag="big")
            for s in range(2):
                nc.tensor.transpose(out=pst[:, s, :],
                                    in_=wt[:, (2 * cp + s) * 128:(2 * cp + s + 1) * 128],
                                    identity=ident[:])
            eng = nc.scalar if (ec + cp) % 2 == 0 else nc.vector
            if eng is nc.scalar:
                nc.scalar.copy(out=wqkvT[:, 2 * cp:2 * cp + 2, ec * 128:(ec + 1) * 128],
                               in_=pst[:])
            else:
                nc.vector.tensor_copy(out=wqkvT[:, 2 * cp:2 * cp + 2, ec * 128:(ec + 1) * 128],
                                      in_=pst[:])

    wprojT = const.tile([128, CC, C], F32R)
    for ec in range(CC):
        wt = tmp.tile([128, C], F32, tag="wload")
        nc.sync.dma_start(out=wt[:], in_=wproj_d[ec * 128:(ec + 1) * 128, :])
        for cp in range(CC // 2):
            pst = ps_big.tile([128, 2, 128], F32, tag="big")
            for s in range(2):
                nc.tensor.transpose(out=pst[:, s, :],
                                    in_=wt[:, (2 * cp + s) * 128:(2 * cp + s + 1) * 128],
                                    identity=ident[:])
            eng = (ec + cp) % 2
            if eng == 0:
                nc.scalar.copy(out=wprojT[:, 2 * cp:2 * cp + 2, ec * 128:(ec + 1) * 128],
                               in_=pst[:])
            else:
                nc.vector.tensor_copy(out=wprojT[:, 2 * cp:2 * cp + 2, ec * 128:(ec + 1) * 128],
                                      in_=pst[:])

    # ---- w_l / w_w transposes; block-diagonal mixers (bf16) ----
    wl_sb = tmp.tile([12, 12], F32, tag="wsml")
    nc.sync.dma_start(out=wl_sb[:], in_=wl_d[:, :])
    ps12 = ps_mix.tile([12, 12], F32, tag="mix")
    nc.tensor.transpose(out=ps12[:], in_=wl_sb[:], identity=ident[:12, :12])
    wlT_plain = const.tile([12, 12], F32)          # w_l^T (for RPE premix)
    nc.scalar.copy(out=wlT_plain[:], in_=ps12[:])
    wlT_scaled = tmp.tile([12, 12], F32, tag="wsml2")
    nc.scalar.mul(out=wlT_scaled[:], in_=ps12[:], mul=SCALE)

    ww_sb = tmp.tile([12, 12], F32, tag="wsml")
    nc.sync.dma_start(out=ww_sb[:], in_=ww_d[:, :])
    ps12b = ps_mix.tile([12, 12], F32, tag="mix")
    nc.tensor.transpose(out=ps12b[:], in_=ww_sb[:], identity=ident[:12, :12])
    wwT = tmp.tile([12, 12], F32, tag="wsml2")
    nc.scalar.copy(out=wwT[:], in_=ps12b[:])

    # nb-major packing: row p = nb*12 + h -> contiguous 12x12 diagonal blocks.
    bd1_f32 = tmp.tile([P96, P96], F32, tag="bd1f")
    nc.vector.memset(bd1_f32[:], 0.0)
    bd2_f32 = tmp.tile([P96, P96], F32, tag="bd2f")
    nc.vector.memset(bd2_f32[:], 0.0)
    for nb in range(NGRP):
        s = nb * H
        nc.gpsimd.dma_start(out=bd1_f32[s:s + H, s:s + H], in_=wlT_scaled[:])
        nc.gpsimd.dma_start(out=bd2_f32[s:s + H, s:s + H], in_=wwT[:])
    bd1 = const.tile([P96, P96], BF16)   # [(nb,h), (nb,g)] = SCALE*w_l[g,h]
    nc.vector.tensor_copy(out=bd1[:], in_=bd1_f32[:])
    bd2 = const.tile([P96, P96], BF16)   # [(nb,g), (nb,h)] = w_w[h,g]
    nc.vector.tensor_copy(out=bd2[:], in_=bd2_f32[:])

    # ---- premixed RPE table: mixed_rpe[g, k] = sum_h w_l[g,h] * rpe[h, k] ----
    rpe_sb = tmp.tile([12, NBKT], F32, tag="rpe")
    nc.sync.dma_start(out=rpe_sb[:], in_=rpe_d[:, :])
    mixed_rpe = tmp.tile([12, NBKT], F32, tag="rpemix")
    for o in range(0, NBKT, 512):
        w = min(512, NBKT - o)
        psr = ps_big.tile([12, 512], F32, tag="big")
        nc.tensor.matmul(out=psr[:, :w], lhsT=wlT_plain[:], rhs=rpe_sb[:, o:o + w],
                         start=True, stop=True)
        nc.scalar.copy(out=mixed_rpe[:, o:o + w], in_=psr[:, :w])

    # replicate across the 8 gather groups: table_rep[16*grp + c] = mixed_rpe[c]
    table_rep = tmp.tile([128, NBKT], F32, tag="trep")
    nc.vector.memset(table_rep[:], 0.0)
    for c in range(12):
        for grp in range(NGRP):
            p = grp * 16 + c
            nc.sync.dma_start(out=table_rep[p:p + 1, :], in_=mixed_rpe[c:c + 1, :])

    # ---- gather indices (wrapped int16 streams per 16-partition group) ----
    rel_flat = rel_d.rearrange("n m -> (n m)")
    idx32 = tmp.tile([128, NIDX // 16], I32, tag="idx32")
    nc.vector.memset(idx32[:], 0)
    for grp in range(NGRP):
        base = grp * NJ * N
        if grp < 7:
            nc.sync.dma_start(
                out=idx32[grp * 16:(grp + 1) * 16, :],
                in_=rel_flat[base:base + NIDX].rearrange("(s p) -> p s", p=16))
        else:
            # group 7 has 28 real n rows (6860 idxs): 16x428 full + 12 tail
            nc.sync.dma_start(
                out=idx32[grp * 16:(grp + 1) * 16, :428],
                in_=rel_flat[base:base + 6848].rearrange("(s p) -> p s", p=16))
            nc.sync.dma_start(
                out=idx32[grp * 16:grp * 16 + 12, 428:429],
                in_=rel_flat[base + 6848:base + 6860].rearrange("(s p) -> p s", p=12))
    idx16 = tmp.tile([128, NIDX // 16], I16, tag="idx16")
    nc.vector.tensor_copy(out=idx16[:], in_=idx32[:])

    # ---- gather premixed bias, then repack to [(nb, h), j*245 + m] ----
    nc.gpsimd.load_library(library_config.ap_gather)
    bias_g = tmp.tile([128, NIDX], F32, tag="biasg")
    nc.gpsimd.ap_gather(
        out_ap=bias_g[:], in_ap=table_rep[:].unsqueeze(2), idxs_ap=idx16[:],
        channels=128, num_elems=NBKT, d=1, num_idxs=NIDX)
    nc.gpsimd.load_library(library_config.standard)

    packed_bias = const.tile([P96, NJ * N], BF16)
    for h in range(12):
        for grp in range(NGRP):
            nc.gpsimd.dma_start(out=packed_bias[grp * H + h:grp * H + h + 1, :],
                                in_=bias_g[grp * 16 + h:grp * 16 + h + 1, :NJ * N])

    # ---- small constants ----
    bw_exp = const.tile([128, CC, 1], F32)   # b_w[(t*128+p)//64]
    for t in range(CC):
        for half in range(2):
            h_idx = 2 * t + half
            nc.gpsimd.dma_start(
                out=bw_exp[half * 64:(half + 1) * 64, t, :],
                in_=bw_d[h_idx:h_idx + 1].unsqueeze(0).to_broadcast([64, 1]))
    bproj_sb = const.tile([128, C], F32)
    nc.gpsimd.dma_start(out=bproj_sb[:], in_=bproj_d[:].unsqueeze(0).to_broadcast([128, C]))
    ones_bf = const.tile([128, 1], BF16)
    nc.vector.memset(ones_bf[:], 1.0)
    zeros_c = const.tile([128, 1], F32)
    nc.vector.memset(zeros_c[:], 0.0)

    ctx0.close()

    # ---- per-batch streaming pools ----
    xb_p = ctx.enter_context(tc.tile_pool(name="xb", bufs=2))
    xT_p = ctx.enter_context(tc.tile_pool(name="xT", bufs=2))
    qT_p = ctx.enter_context(tc.tile_pool(name="qT", bufs=2))
    kT_p = ctx.enter_context(tc.tile_pool(name="kT", bufs=2))
    v_p = ctx.enter_context(tc.tile_pool(name="v", bufs=2))
    swt_p = ctx.enter_context(tc.tile_pool(name="swt", bufs=2))
    pk_p = ctx.enter_context(tc.tile_pool(name="pk", bufs=3))
    et_p = ctx.enter_context(tc.tile_pool(name="et", bufs=4))
    bdj_p = ctx.enter_context(tc.tile_pool(name="bdj", bufs=4))
    at_p = ctx.enter_context(tc.tile_pool(name="at", bufs=2))
    oT_p = ctx.enter_context(tc.tile_pool(name="oT", bufs=2))
    y_p = ctx.enter_context(tc.tile_pool(name="y", bufs=2))
    st_p = ctx.enter_context(tc.tile_pool(name="st", bufs=4))

    STOP = os.environ.get("K_STOP", "full")
    for b in range(BLOC):
        # ---- load x_b and transpose to xT [c, n] (f32r, n padded to 256) ----
        xb = xb_p.tile([128, 2, C], F32)
        for mc, (mo, msz) in enumerate(MCS):
            nc.sync.dma_start(out=xb[:msz, mc, :], in_=x_d[b, mo:mo + msz, :])
        xT = xT_p.tile([128, CC, NPAD], F32R)
        nc.vector.tensor_copy(out=xT[:, :, N:],
                              in_=zeros_c[:, 0:1].to_broadcast([128, CC, NPAD - N]))
        for mc, (mo, msz) in enumerate(MCS):
            for cp in range(CC // 2):
                pst = ps_big.tile([128, 2, 128], F32, tag="big")
                for s in range(2):
                    nc.tensor.transpose(
                        out=pst[:, s, :msz],
                        in_=xb[:msz, mc, (2 * cp + s) * 128:(2 * cp + s + 1) * 128],
                        identity=ident[:msz, :msz])
                nc.scalar.copy(out=xT[:, 2 * cp:2 * cp + 2, mo:mo + msz],
                               in_=pst[:, :, :msz])

        # ---- QKV: q,k -> bf16 [hd, n]; v -> bf16 [m, hd] ----
        qT = qT_p.tile([128, CC, NPAD], BF16)
        kT = kT_p.tile([128, CC, NPAD], BF16)
        for ep in range(6):           # ec pairs: ep<3 -> q chunks, else k chunks
            psq = ps_big.tile([128, 2, NPAD], F32, tag="big")
            for s in range(2):
                ec = 2 * ep + s
                for cc in range(CC):
                    nc.tensor.matmul(out=psq[:, s, :],
                                     lhsT=wqkvT[:, cc, ec * 128:(ec + 1) * 128],
                                     rhs=xT[:, cc, :],
                                     start=(cc == 0), stop=(cc == CC - 1))
            dst = qT if ep < 3 else kT
            do = (2 * ep) % CC
            if ep % 2 == 0:
                nc.scalar.copy(out=dst[:, do:do + 2, :], in_=psq[:])
            else:
                nc.vector.tensor_copy(out=dst[:, do:do + 2, :], in_=psq[:])

        v_sb = v_p.tile([128, 2, C], BF16)        # [m, (h,d)]
        for mc, (mo, msz) in enumerate(MCS):
            for vc in range(2):
                psv = ps_big.tile([128, 384], F32, tag="big")
                for cc in range(CC):
                    nc.tensor.matmul(
                        out=psv[:msz], lhsT=xT[:, cc, mo:mo + msz],
                        rhs=wqkvT[:, cc, 2 * C + vc * 384:2 * C + (vc + 1) * 384],
                        start=(cc == 0), stop=(cc == CC - 1))
                nc.vector.tensor_copy(out=v_sb[:msz, mc, vc * 384:(vc + 1) * 384],
                                      in_=psv[:msz])

        if STOP == "qkv":
            continue
        # ---- b_w * colsum(v) ----
        SKIP_BWV = os.environ.get("K_SKIP_BWV")
        bwv = st_p.tile([128, CC, 1], F32, tag="bwv", bufs=2)
        if SKIP_BWV:
            nc.vector.memset(bwv[:], 0.0)
        for t in range(CC if not SKIP_BWV else 0):
            psvs = ps_at.tile([128, 1], F32, tag="at")
            for mc, (mo, msz) in enumerate(MCS):
                nc.tensor.matmul(out=psvs[:], lhsT=v_sb[:msz, mc, t * 128:(t + 1) * 128],
                                 rhs=ones_bf[:msz, :],
                                 start=(mc == 0), stop=(mc == 1))
            nc.vector.tensor_tensor(out=bwv[:, t, :], in0=psvs[:], in1=bw_exp[:, t, :], op=MULT)

        # ---- QK^T -> swt [m, (mc, j, nb, h)] bf16 ----
        # heads paired same-parity (h, h+2): lhsT partition offset must be
        # constant across matmuls that share a PSUM bank (HW constraint)
        swt = swt_p.tile([128, 2, NJ, NGRP, H], BF16)
        for mc, (mo, msz) in enumerate(MCS):
            for parity in range(2):
                for tp in range(3):
                    pss = ps_big.tile([128, 2, NPAD], F32, tag="big")
                    po = parity * 64
                    for s in range(2):
                        t = 2 * tp + s
                        nc.tensor.matmul(
                            out=pss[:msz, s, :],
                            lhsT=kT[po:po + 64, t, mo:mo + msz],
                            rhs=qT[po:po + 64, t, :],
                            start=True, stop=True)
                    src = pss[:msz, :, :NJP].rearrange("p h (nb j) -> p j nb h", j=NJ)
                    ha = 4 * tp + parity     # h = 2t + parity -> ha, ha+2
                    if (parity + tp) % 2 == 0:
                        nc.scalar.copy(out=swt[:msz, mc, :, :, ha:ha + 3:2], in_=src)
                    else:
                        nc.vector.tensor_copy(out=swt[:msz, mc, :, :, ha:ha + 3:2], in_=src)

        if STOP == "qk":
            continue
        # ---- per-j packed attention (j processed in pairs) ----
        atw = at_p.tile([128, 2, H, NJP], BF16)   # cols j-major: col = j*8 + nb
        for jp in range((NJ + 1) // 2):
            js = [2 * jp] + ([2 * jp + 1] if 2 * jp + 1 < NJ else [])
            # T1: packed S^T [(nb,h), m] for both j of the pair -> one bank
            pspk = ps_t1.tile([P96, 2, NPAD], BF16, tag="t1")
            for ji, j in enumerate(js):
                for mc, (mo, msz) in enumerate(MCS):
                    nc.tensor.transpose(
                        out=pspk[:, ji, mo:mo + msz],
                        in_=swt[:msz, mc, j, :, :].rearrange("p nb h -> p (nb h)"),
                        identity=ident_bf[:msz, :msz])
            pk = pk_p.tile([P96, 2, NPAD], BF16, tag="pk")
            nc.vector.tensor_copy(out=pk[:, :len(js), :N], in_=pspk[:, :len(js), :N])

            ssum = st_p.tile([P96, 2], F32, tag="ss")
            ets = []
            for ji, j in enumerate(js):
                # premix (block-diag) + RPE bias, both as matmuls into PSUM
                psm = ps_mix.tile([P96, NPAD], F32, tag="mix")
                nc.tensor.matmul(out=psm[:, :N], lhsT=bd1[:], rhs=pk[:, ji, :N],
                                 start=True, stop=False)
                nc.tensor.matmul(out=psm[:, :N], lhsT=ident_bf[:P96, :P96],
                                 rhs=packed_bias[:, j * N:(j + 1) * N],
                                 start=False, stop=True)
                et = et_p.tile([P96, NPAD], BF16, tag="et")
                # zero the pad cols so the mc1 postmix can emit full 128 rows
                nc.gpsimd.memset(et[:, N:], 0.0)
                nc.scalar.activation(out=et[:, :N], in_=psm[:, :N], func=EXP,
                                     scale=1.0, accum_out=ssum[:, ji:ji + 1])
                ets.append(et)
            rec = st_p.tile([P96, 2], F32, tag="rc")
            nc.vector.reciprocal(out=rec[:, :len(js)], in_=ssum[:, :len(js)])

            psat = ps_at.tile([128, 2, 2, P96], F32, tag="at")  # [m, (jj, mc), 96]
            for ji, j in enumerate(js):
                bd2j = bdj_p.tile([P96, P96], BF16, tag="bdj")
                nc.gpsimd.tensor_scalar_mul(bd2j[:], bd2[:], rec[:, ji:ji + 1])
                for mc in range(2):
                    # mc1 uses a full 128-col lhsT slice (real m + zeroed pad)
                    nc.tensor.matmul(out=psat[:, ji, mc, :],
                                     lhsT=ets[ji][:, mc * 128:mc * 128 + 128],
                                     rhs=bd2j[:],
                                     start=True, stop=True)
            for ji, j in enumerate(js):
                src = psat[:, ji, :, :].rearrange("p m (n h) -> p m h n", n=NGRP)
                if jp % 2 == 0:
                    nc.vector.tensor_copy(
                        out=atw[:, :, :, j * NGRP:(j + 1) * NGRP], in_=src)
                else:
                    nc.scalar.copy(
                        out=atw[:, :, :, j * NGRP:(j + 1) * NGRP], in_=src)

        if STOP == "jloop":
            continue
        # ---- AV (+ b_w colsum term) -> outT [(h,d), n] f32r (n order) ----
        outT = oT_p.tile([128, CC, NJP], F32R)
        for half in range(2):
            for tp in range(3):
                psav = ps_mix.tile([64, 2, NPAD], F32, tag="mix")
                for s in range(2):
                    h = 4 * tp + 2 * s + half
                    for mc, (mo, msz) in enumerate(MCS):
                        nc.tensor.matmul(out=psav[:, s, :NJP],
                                         lhsT=v_sb[:msz, mc, h * 64:(h + 1) * 64],
                                         rhs=atw[:msz, mc, h, :],
                                         start=(mc == 0), stop=(mc == 1))
                for s in range(2):
                    t = 2 * tp + s
                    nc.scalar.activation(
                        out=outT[half * 64:(half + 1) * 64, t, :].rearrange(
                            "p (nb j) -> p nb j", j=NJ),
                        in_=psav[:64, s, :NJP].rearrange("p (j nb) -> p nb j", nb=NGRP),
                        func=IDENT, bias=bwv[half * 64:(half + 1) * 64, t, :], scale=1.0)

        if STOP == "av":
            continue
        # ---- projection + b_proj -> y -> DRAM ----
        for mc, (mo, msz) in enumerate(MCS):
            y = y_p.tile([128, C], F32)
            for half in range(2):
                psy = ps_big.tile([128, 384], F32, tag="big")
                for cc in range(CC):
                    nc.tensor.matmul(
                        out=psy[:msz], lhsT=outT[:, cc, mo:mo + msz],
                        rhs=wprojT[:, cc, half * 384:(half + 1) * 384],
                        start=(cc == 0), stop=(cc == CC - 1))
                nc.vector.tensor_tensor(out=y[:msz, half * 384:(half + 1) * 384],
                                        in0=psy[:msz],
                                        in1=bproj_sb[:msz, half * 384:(half + 1) * 384],
                                        op=ADD)
            nc.sync.dma_start(out=out_d[b, mo:mo + msz, :], in_=y[:msz, :])


_CACHE = {}


def _build():
    if "nc" in _CACHE:
        return _CACHE["nc"]
    nc = bacc.Bacc("TRN2", target_bir_lowering=False, debug=False, num_devices=NCORES)
    io = (
        nc.dram_tensor("x", [BLOC, N, C], F32, kind="ExternalInput").ap(),
        nc.dram_tensor("w_qkv", [E, C], F32, kind="ExternalInput").ap(),
        nc.dram_tensor("w_proj", [C, C], F32, kind="ExternalInput").ap(),
        nc.dram_tensor("b_proj", [C], F32, kind="ExternalInput").ap(),
        nc.dram_tensor("w_l", [H, H], F32, kind="ExternalInput").ap(),
        nc.dram_tensor("w_w", [H, H], F32, kind="ExternalInput").ap(),
        nc.dram_tensor("b_w", [H], F32, kind="ExternalInput").ap(),
        nc.dram_tensor("rpe_table", [H, NBKT], F32, kind="ExternalInput").ap(),
        nc.dram_tensor("rel_idx", [N, N], I32, kind="ExternalInput").ap(),
        nc.dram_tensor("out", [BLOC, N, C], F32, kind="ExternalOutput").ap(),
    )
    with tile.TileContext(nc) as tc, ExitStack() as ctx:
        _emit(ctx, tc, io)
    nc.compile()
    _CACHE["nc"] = nc
    return nc


def kernel(x, w_qkv, w_proj, b_proj, w_l, b_l, w_w, b_w, rpe_table, rel_idx,
           _trace=False):
    nc = _build()
    shared = {
        "w_qkv": np.ascontiguousarray(w_qkv, np.float32),
        "w_proj": np.ascontiguousarray(w_proj, np.float32),
        "b_proj": np.ascontiguousarray(b_proj, np.float32),
        "w_l": np.ascontiguousarray(w_l, np.float32),
        "w_w": np.ascontiguousarray(w_w, np.float32),
        "b_w": np.ascontiguousarray(b_w, np.float32),
        "rpe_table": np.ascontiguousarray(rpe_table, np.float32),
        "rel_idx": np.ascontiguousarray(rel_idx, np.int32),
    }
    x = np.ascontiguousarray(x, np.float32)
    in_maps = [dict(shared, x=x[i * BLOC:(i + 1) * BLOC]) for i in range(NCORES)]
    res = run_bass_kernel_spmd(nc, in_maps, core_ids=list(range(NCORES)),
                               trace=_trace)
    out = np.concatenate([res.results[i]["out"] for i in range(NCORES)], axis=0)
    if _trace:
        kernel.last_result = res
    return out


# revision 11
# speedup vs baseline: 1.7496x; 1.5262x over previous
"""Talking-heads attention Trainium2 kernel (Bass/Tile), 8-core data-parallel.

Problem: nn_Attention_talking_head — B=64, N=245, C=768, H=12, D=64,
RPE table (12, 1698) indexed by rel_idx (245, 245), talking-heads mixing
(12x12) before and after softmax, in/out projections.

Sharding: batch 64 -> 8 cores x 8 batches. Weights replicated. No collectives.

v2 design notes (vs v1):
  - bf16 attention pipeline (qT/kT/swt/pk/et/atw/v_sb + mixers); f32r GEMM
    anchors for QKV-from-x and the output projection.
  - max-subtraction dropped (|logit| <= ~2.5 for this problem's scales).
  - RPE bias add folded into the premix as a second accumulating matmul
    (identity lhsT), so softmax reads PSUM directly.
  - softmax normalization folded into the postmix by row-scaling the small
    w_w block-diagonal mixer with 1/rowsum (Pool engine).
  - per-tag PSUM pools (big/t1/mix/at = 2+2+2+2 banks) so consecutive j
    iterations pipeline instead of serializing on bank reuse.
  - paired evictions: QKV 2 e-chunks/bank, QK^T 2 heads/bank, T1 2 j/bank,
    postmix 2 j/bank -> fewer fixed-cost DVE/Act instructions.
  - atw uses j-major column order (col = j*8+nb); the AV eviction un-permutes
    back to n order via strided views.

b_l is mathematically a no-op (constant per softmax row) and is skipped.
"""
import os
import numpy as np
from contextlib import ExitStack

import concourse.bass as bass
import concourse.tile as tile
from concourse import bacc, mybir, library_config
from concourse.bass_utils import run_bass_kernel_spmd
from concourse.masks import make_identity

F32 = mybir.dt.float32
F32R = mybir.dt.float32r
BF16 = mybir.dt.bfloat16
I32 = mybir.dt.int32
I16 = mybir.dt.int16
AX = mybir.AxisListType.X
EXP = mybir.ActivationFunctionType.Exp
IDENT = mybir.ActivationFunctionType.Identity
ADD = mybir.AluOpType.add
MULT = mybir.AluOpType.mult

NCORES = 8
B, N, C, H, D = 64, 245, 768, 12, 64
BLOC = B // NCORES          # 8 batches per core
E = 3 * C                   # 2304
NBKT = 1698
SCALE = D ** -0.5
NPAD = 256
NGRP = 8                    # gather groups == packed nb slots
NJ = 31                     # packed tiles per batch; n = 31*nb + j, j in [0, NJ)
NJP = NJ * NGRP             # 248 packed col slots
NIDX = 7600                 # gather stream length per group (31*245 real + 5 pad)
CC = C // 128               # 6 contraction chunks
MCS = [(0, 128), (128, 117)]
P96 = H * NGRP              # 96 packed rows


def _emit(ctx: ExitStack, tc, io):
    nc = tc.nc
    x_d, wqkv_d, wproj_d, bproj_d, wl_d, ww_d, bw_d, rpe_d, rel_d, out_d = io

    const = ctx.enter_context(tc.tile_pool(name="const", bufs=1))
    ctx0 = ctx.enter_context(ExitStack())
    tmp = ctx0.enter_context(tc.tile_pool(name="tmp", bufs=1))
    ps_big = ctx.enter_context(tc.tile_pool(name="ps_big", bufs=2, space="PSUM"))
    ps_t1 = ctx.enter_context(tc.tile_pool(name="ps_t1", bufs=2, space="PSUM"))
    ps_mix = ctx.enter_context(tc.tile_pool(name="ps_mix", bufs=2, space="PSUM"))
    ps_at = ctx.enter_context(tc.tile_pool(name="ps_at", bufs=2, space="PSUM"))

    ident = const.tile([128, 128], F32)
    make_identity(nc, ident[:])
    ident_bf = const.tile([128, 128], BF16)
    nc.vector.tensor_copy(out=ident_bf[:], in_=ident[:])

    # ---- weight transposes (PE), paired evictions ----
    wqkvT = const.tile([128, CC, E], F32R)   # [c-part, c-chunk, e]
    for ec in range(E // 128):
        wt = tmp.tile([128, C], F32, tag="wload")
        nc.sync.dma_start(out=wt[:], in_=wqkv_d[ec * 128:(ec + 1) * 128, :])
        for cp in range(CC // 2):
            pst = ps_big.tile([128, 2, 128], F32, tag="big")
            for s in range(2):
                nc.tensor.transpose(out=pst[:, s, :],
                                    in_=wt[:, (2 * cp + s) * 128:(2 * cp + s + 1) * 128],
                                    identity=ident[:])
            eng = nc.scalar if (ec + cp) % 2 == 0 else nc.vector
            if eng is nc.scalar:
                nc.scalar.copy(out=wqkvT[:, 2 * cp:2 * cp + 2, ec * 128:(ec + 1) * 128],
                               in_=pst[:])
            else:
                nc.vector.tensor_copy(out=wqkvT[:, 2 * cp:2 * cp + 2, ec * 128:(ec + 1) * 128],
                                      in_=pst[:])

    wprojT = const.tile([128, CC, C], F32R)
    for ec in range(CC):
        wt = tmp.tile([128, C], F32, tag="wload")
        nc.sync.dma_start(out=wt[:], in_=wproj_d[ec * 128:(ec + 1) * 128, :])
        for cp in range(CC // 2):
            pst = ps_big.tile([128, 2, 128], F32, tag="big")
            for s in range(2):
                nc.tensor.transpose(out=pst[:, s, :],
                                    in_=wt[:, (2 * cp + s) * 128:(2 * cp + s + 1) * 128],
                                    identity=ident[:])
            eng = (ec + cp) % 2
            if eng == 0:
                nc.scalar.copy(out=wprojT[:, 2 * cp:2 * cp + 2, ec * 128:(ec + 1) * 128],
                               in_=pst[:])
            else:
                nc.vector.tensor_copy(out=wprojT[:, 2 * cp:2 * cp + 2, ec * 128:(ec + 1) * 128],
                                      in_=pst[:])

    # ---- w_l / w_w transposes; block-diagonal mixers (bf16) ----
    wl_sb = tmp.tile([12, 12], F32, tag="wsml")
    nc.sync.dma_start(out=wl_sb[:], in_=wl_d[:, :])
    ps12 = ps_mix.tile([12, 12], F32, tag="mix")
    nc.tensor.transpose(out=ps12[:], in_=wl_sb[:], identity=ident[:12, :12])
    wlT_plain = const.tile([12, 12], F32)          # w_l^T (for RPE premix)
    nc.scalar.copy(out=wlT_plain[:], in_=ps12[:])
    wlT_scaled = tmp.tile([12, 12], F32, tag="wsml2")
    nc.scalar.mul(out=wlT_scaled[:], in_=ps12[:], mul=SCALE)

    ww_sb = tmp.tile([12, 12], F32, tag="wsml")
    nc.sync.dma_start(out=ww_sb[:], in_=ww_d[:, :])
    ps12b = ps_mix.tile([12, 12], F32, tag="mix")
    nc.tensor.transpose(out=ps12b[:], in_=ww_sb[:], identity=ident[:12, :12])
    wwT = tmp.tile([12, 12], F32, tag="wsml2")
    nc.scalar.copy(out=wwT[:], in_=ps12b[:])

    # nb-major packing: row p = nb*12 + h -> contiguous 12x12 diagonal blocks.
    bd1_f32 = tmp.tile([P96, P96], F32, tag="bd1f")
    nc.vector.memset(bd1_f32[:], 0.0)
    bd2_f32 = tmp.tile([P96, P96], F32, tag="bd2f")
    nc.vector.memset(bd2_f32[:], 0.0)
    for nb in range(NGRP):
        s = nb * H
        nc.gpsimd.dma_start(out=bd1_f32[s:s + H, s:s + H], in_=wlT_scaled[:])
        nc.gpsimd.dma_start(out=bd2_f32[s:s + H, s:s + H], in_=wwT[:])
    bd1 = const.tile([P96, P96], BF16)   # [(nb,h), (nb,g)] = SCALE*w_l[g,h]
    nc.vector.tensor_copy(out=bd1[:], in_=bd1_f32[:])
    bd2 = const.tile([P96, P96], BF16)   # [(nb,g), (nb,h)] = w_w[h,g]
    nc.vector.tensor_copy(out=bd2[:], in_=bd2_f32[:])

    # ---- premixed RPE table: mixed_rpe[g, k] = sum_h w_l[g,h] * rpe[h, k] ----
    rpe_sb = tmp.tile([12, NBKT], F32, tag="rpe")
    nc.sync.dma_start(out=rpe_sb[:], in_=rpe_d[:, :])
    mixed_rpe = tmp.tile([12, NBKT], F32, tag="rpemix")
    for o in range(0, NBKT, 512):
        w = min(512, NBKT - o)
        psr = ps_big.tile([12, 512], F32, tag="big")
        nc.tensor.matmul(out=psr[:, :w], lhsT=wlT_plain[:], rhs=rpe_sb[:, o:o + w],
                         start=True, stop=True)
        nc.scalar.copy(out=mixed_rpe[:, o:o + w], in_=psr[:, :w])

    # replicate across the 8 gather groups: table_rep[16*grp + c] = mixed_rpe[c]
    table_rep = tmp.tile([128, NBKT], F32, tag="trep")
    nc.vector.memset(table_rep[:], 0.0)
    for c in range(12):
        for grp in range(NGRP):
            p = grp * 16 + c
            nc.sync.dma_start(out=table_rep[p:p + 1, :], in_=mixed_rpe[c:c + 1, :])

    # ---- gather indices (wrapped int16 streams per 16-partition group) ----
    rel_flat = rel_d.rearrange("n m -> (n m)")
    idx32 = tmp.tile([128, NIDX // 16], I32, tag="idx32")
    nc.vector.memset(idx32[:], 0)
    for grp in range(NGRP):
        base = grp * NJ * N
        if grp < 7:
            nc.sync.dma_start(
                out=idx32[grp * 16:(grp + 1) * 16, :],
                in_=rel_flat[base:base + NIDX].rearrange("(s p) -> p s", p=16))
        else:
            # group 7 has 28 real n rows (6860 idxs): 16x428 full + 12 tail
            nc.sync.dma_start(
                out=idx32[grp * 16:(grp + 1) * 16, :428],
                in_=rel_flat[base:base + 6848].rearrange("(s p) -> p s", p=16))
            nc.sync.dma_start(
                out=idx32[grp * 16:grp * 16 + 12, 428:429],
                in_=rel_flat[base + 6848:base + 6860].rearrange("(s p) -> p s", p=12))
    idx16 = tmp.tile([128, NIDX // 16], I16, tag="idx16")
    nc.vector.tensor_copy(out=idx16[:], in_=idx32[:])

    # ---- gather premixed bias, then repack to [(nb, h), j*245 + m] ----
    nc.gpsimd.load_library(library_config.ap_gather)
    bias_g = tmp.tile([128, NIDX], F32, tag="biasg")
    nc.gpsimd.ap_gather(
        out_ap=bias_g[:], in_ap=table_rep[:].unsqueeze(2), idxs_ap=idx16[:],
        channels=128, num_elems=NBKT, d=1, num_idxs=NIDX)
    nc.gpsimd.load_library(library_config.standard)

    packed_bias = const.tile([P96, NJ * N], BF16)
    for h in range(12):
        for grp in range(NGRP):
            nc.gpsimd.dma_start(out=packed_bias[grp * H + h:grp * H + h + 1, :],
                                in_=bias_g[grp * 16 + h:grp * 16 + h + 1, :NJ * N])

    # ---- small constants ----
    bw_exp = const.tile([128, CC, 1], F32)   # b_w[(t*128+p)//64]
    for t in range(CC):
        for half in range(2):
            h_idx = 2 * t + half
            nc.gpsimd.dma_start(
                out=bw_exp[half * 64:(half + 1) * 64, t, :],
                in_=bw_d[h_idx:h_idx + 1].unsqueeze(0).to_broadcast([64, 1]))
    bproj_sb = const.tile([128, C], F32)
    nc.gpsimd.dma_start(out=bproj_sb[:], in_=bproj_d[:].unsqueeze(0).to_broadcast([128, C]))
    ones_bf = const.tile([128, 1], BF16)
    nc.vector.memset(ones_bf[:], 1.0)
    zeros_c = const.tile([128, 1], F32)
    nc.vector.memset(zeros_c[:], 0.0)

    ctx0.close()

    # ---- per-batch streaming pools ----
    xb_p = ctx.enter_context(tc.tile_pool(name="xb", bufs=2))
    xT_p = ctx.enter_context(tc.tile_pool(name="xT", bufs=2))
    qT_p = ctx.enter_context(tc.tile_pool(name="qT", bufs=2))
    kT_p = ctx.enter_context(tc.tile_pool(name="kT", bufs=2))
    v_p = ctx.enter_context(tc.tile_pool(name="v", bufs=2))
    swt_p = ctx.enter_context(tc.tile_pool(name="swt", bufs=2))
    pk_p = ctx.enter_context(tc.tile_pool(name="pk", bufs=3))
    et_p = ctx.enter_context(tc.tile_pool(name="et", bufs=4))
    bdj_p = ctx.enter_context(tc.tile_pool(name="bdj", bufs=4))
    at_p = ctx.enter_context(tc.tile_pool(name="at", bufs=2))
    oT_p = ctx.enter_context(tc.tile_pool(name="oT", bufs=2))
    y_p = ctx.enter_context(tc.tile_pool(name="y", bufs=2))
    st_p = ctx.enter_context(tc.tile_pool(name="st", bufs=4))

    STOP = os.environ.get("K_STOP", "full")
    for b in range(BLOC):
        # ---- load x_b and transpose to xT [c, n] (f32r, n padded to 256) ----
        xb = xb_p.tile([128, 2, C], F32)
        for mc, (mo, msz) in enumerate(MCS):
            nc.sync.dma_start(out=xb[:msz, mc, :], in_=x_d[b, mo:mo + msz, :])
        xT = xT_p.tile([128, CC, NPAD], F32R)
        nc.vector.tensor_copy(out=xT[:, :, N:],
                              in_=zeros_c[:, 0:1].to_broadcast([128, CC, NPAD - N]))
        for mc, (mo, msz) in enumerate(MCS):
            for cp in range(CC // 2):
                pst = ps_big.tile([128, 2, 128], F32, tag="big")
                for s in range(2):
                    nc.tensor.transpose(
                        out=pst[:, s, :msz],
                        in_=xb[:msz, mc, (2 * cp + s) * 128:(2 * cp + s + 1) * 128],
                        identity=ident[:msz, :msz])
                nc.scalar.copy(out=xT[:, 2 * cp:2 * cp + 2, mo:mo + msz],
                               in_=pst[:, :, :msz])

        # ---- QKV: q,k -> bf16 [hd, n]; v -> bf16 [m, hd] ----
        qT = qT_p.tile([128, CC, NPAD], BF16)
        kT = kT_p.tile([128, CC, NPAD], BF16)
        for ep in range(6):           # ec pairs: ep<3 -> q chunks, else k chunks
            psq = ps_big.tile([128, 2, NPAD], F32, tag="big")
            for s in range(2):
                ec = 2 * ep + s
                for cc in range(CC):
                    nc.tensor.matmul(out=psq[:, s, :],
                                     lhsT=wqkvT[:, cc, ec * 128:(ec + 1) * 128],
                                     rhs=xT[:, cc, :],
                                     start=(cc == 0), stop=(cc == CC - 1))
            dst = qT if ep < 3 else kT
            do = (2 * ep) % CC
            if ep % 2 == 0:
                nc.scalar.copy(out=dst[:, do:do + 2, :], in_=psq[:])
            else:
                nc.vector.tensor_copy(out=dst[:, do:do + 2, :], in_=psq[:])

        v_sb = v_p.tile([128, 2, C], BF16)        # [m, (h,d)]
        for mc, (mo, msz) in enumerate(MCS):
            for vc in range(2):
                psv = ps_big.tile([128, 384], F32, tag="big")
                for cc in range(CC):
                    nc.tensor.matmul(
                        out=psv[:msz], lhsT=xT[:, cc, mo:mo + msz],
                        rhs=wqkvT[:, cc, 2 * C + vc * 384:2 * C + (vc + 1) * 384],
                        start=(cc == 0), stop=(cc == CC - 1))
                nc.vector.tensor_copy(out=v_sb[:msz, mc, vc * 384:(vc + 1) * 384],
                                      in_=psv[:msz])

        if STOP == "qkv":
            continue
        # ---- b_w * colsum(v) ----
        SKIP_BWV = os.environ.get("K_SKIP_BWV")
        bwv = st_p.tile([128, CC, 1], F32, tag="bwv", bufs=2)
        if SKIP_BWV:
            nc.vector.memset(bwv[:], 0.0)
        for t in range(CC if not SKIP_BWV else 0):
            psvs = ps_at.tile([128, 1], F32, tag="at")
            for mc, (mo, msz) in enumerate(MCS):
                nc.tensor.matmul(out=psvs[:], lhsT=v_sb[:msz, mc, t * 128:(t + 1) * 128],
                                 rhs=ones_bf[:msz, :],
                                 start=(mc == 0), stop=(mc == 1))
            nc.vector.tensor_tensor(out=bwv[:, t, :], in0=psvs[:], in1=bw_exp[:, t, :], op=MULT)

        # ---- QK^T -> swt [m, (mc, j, nb, h)] bf16 ----
        # heads paired same-parity (h, h+2): lhsT partition offset must be
        # constant across matmuls that share a PSUM bank (HW constraint)
        swt = swt_p.tile([128, 2, NJ, NGRP, H], BF16)
        for mc, (mo, msz) in enumerate(MCS):
            for parity in range(2):
                for tp in range(3):
                    pss = ps_big.tile([128, 2, NPAD], F32, tag="big")
                    po = parity * 64
                    for s in range(2):
                        t = 2 * tp + s
                        nc.tensor.matmul(
                            out=pss[:msz, s, :],
                            lhsT=kT[po:po + 64, t, mo:mo + msz],
                            rhs=qT[po:po + 64, t, :],
                            start=True, stop=True)
                    src = pss[:msz, :, :NJP].rearrange("p h (nb j) -> p j nb h", j=NJ)
                    ha = 4 * tp + parity     # h = 2t + parity -> ha, ha+2
                    if (parity + tp) % 2 == 0:
                        nc.scalar.copy(out=swt[:msz, mc, :, :, ha:ha + 3:2], in_=src)
                    else:
                        nc.vector.tensor_copy(out=swt[:msz, mc, :, :, ha:ha + 3:2], in_=src)

        if STOP == "qk":
            continue
        # ---- per-j packed attention (j processed in pairs) ----
        atw = at_p.tile([128, 2, H, NJP], BF16)   # cols j-major: col = j*8 + nb
        for jp in range((NJ + 1) // 2):
            js = [2 * jp] + ([2 * jp + 1] if 2 * jp + 1 < NJ else [])
            # T1: packed S^T [(nb,h), m] for both j of the pair -> one bank
            pspk = ps_t1.tile([P96, 2, NPAD], BF16, tag="t1")
            for ji, j in enumerate(js):
                for mc, (mo, msz) in enumerate(MCS):
                    nc.tensor.transpose(
                        out=pspk[:, ji, mo:mo + msz],
                        in_=swt[:msz, mc, j, :, :].rearrange("p nb h -> p (nb h)"),
                        identity=ident_bf[:msz, :msz])
            pk = pk_p.tile([P96, 2, NPAD], BF16, tag="pk")
            nc.vector.tensor_copy(out=pk[:, :len(js), :N], in_=pspk[:, :len(js), :N])

            ssum = st_p.tile([P96, 2], F32, tag="ss")
            ets = []
            for ji, j in enumerate(js):
                # premix (block-diag) + RPE bias, both as matmuls into PSUM
                psm = ps_mix.tile([P96, NPAD], F32, tag="mix")
                nc.tensor.matmul(out=psm[:, :N], lhsT=bd1[:], rhs=pk[:, ji, :N],
                                 start=True, stop=False)
                nc.tensor.matmul(out=psm[:, :N], lhsT=ident_bf[:P96, :P96],
                                 rhs=packed_bias[:, j * N:(j + 1) * N],
                                 start=False, stop=True)
                et = et_p.tile([P96, NPAD], BF16, tag="et")
                # zero the pad cols so the mc1 postmix can emit full 128 rows
                nc.gpsimd.memset(et[:, N:], 0.0)
                nc.scalar.activation(out=et[:, :N], in_=psm[:, :N], func=EXP,
                                     scale=1.0, accum_out=ssum[:, ji:ji + 1])
                ets.append(et)
            rec = st_p.tile([P96, 2], F32, tag="rc")
            nc.vector.reciprocal(out=rec[:, :len(js)], in_=ssum[:, :len(js)])

            psat = ps_at.tile([128, 2, 2, P96], F32, tag="at")  # [m, (jj, mc), 96]
            for ji, j in enumerate(js):
                bd2j = bdj_p.tile([P96, P96], BF16, tag="bdj")
                nc.vector.tensor_scalar_mul(bd2j[:], bd2[:], rec[:, ji:ji + 1])
                for mc in range(2):
                    # mc1 uses a full 128-col lhsT slice (real m + zeroed pad)
                    nc.tensor.matmul(out=psat[:, ji, mc, :],
                                     lhsT=ets[ji][:, mc * 128:mc * 128 + 128],
                                     rhs=bd2j[:],
                                     start=True, stop=True)
            for ji, j in enumerate(js):
                src = psat[:, ji, :, :].rearrange("p m (n h) -> p m h n", n=NGRP)
                if jp % 2 == 0:
                    nc.vector.tensor_copy(
                        out=atw[:, :, :, j * NGRP:(j + 1) * NGRP], in_=src)
                else:
                    nc.scalar.copy(
                        out=atw[:, :, :, j * NGRP:(j + 1) * NGRP], in_=src)

        if STOP == "jloop":
            continue
        # ---- AV (+ b_w colsum term) -> outT [(h,d), n] f32r (n order) ----
        outT = oT_p.tile([128, CC, NJP], F32R)
        for half in range(2):
            for tp in range(3):
                psav = ps_mix.tile([64, 2, NPAD], F32, tag="mix")
                for s in range(2):
                    h = 4 * tp + 2 * s + half
                    for mc, (mo, msz) in enumerate(MCS):
                        nc.tensor.matmul(out=psav[:, s, :NJP],
                                         lhsT=v_sb[:msz, mc, h * 64:(h + 1) * 64],
                                         rhs=atw[:msz, mc, h, :],
                                         start=(mc == 0), stop=(mc == 1))
                for s in range(2):
                    t = 2 * tp + s
                    nc.scalar.activation(
                        out=outT[half * 64:(half + 1) * 64, t, :].rearrange(
                            "p (nb j) -> p nb j", j=NJ),
                        in_=psav[:64, s, :NJP].rearrange("p (j nb) -> p nb j", nb=NGRP),
                        func=IDENT, bias=bwv[half * 64:(half + 1) * 64, t, :], scale=1.0)

        if STOP == "av":
            continue
        # ---- projection + b_proj -> y -> DRAM ----
        for mc, (mo, msz) in enumerate(MCS):
            y = y_p.tile([128, C], F32)
            for half in range(2):
                psy = ps_big.tile([128, 384], F32, tag="big")
                for cc in range(CC):
                    nc.tensor.matmul(
                        out=psy[:msz], lhsT=outT[:, cc, mo:mo + msz],
                        rhs=wprojT[:, cc, half * 384:(half + 1) * 384],
                        start=(cc == 0), stop=(cc == CC - 1))
                nc.vector.tensor_tensor(out=y[:msz, half * 384:(half + 1) * 384],
                                        in0=psy[:msz],
                                        in1=bproj_sb[:msz, half * 384:(half + 1) * 384],
                                        op=ADD)
            nc.sync.dma_start(out=out_d[b, mo:mo + msz, :], in_=y[:msz, :])


_CACHE = {}


def _build():
    if "nc" in _CACHE:
        return _CACHE["nc"]
    nc = bacc.Bacc("TRN2", target_bir_lowering=False, debug=False, num_devices=NCORES)
    io = (
        nc.dram_tensor("x", [BLOC, N, C], F32, kind="ExternalInput").ap(),
        nc.dram_tensor("w_qkv", [E, C], F32, kind="ExternalInput").ap(),
        nc.dram_tensor("w_proj", [C, C], F32, kind="ExternalInput").ap(),
        nc.dram_tensor("b_proj", [C], F32, kind="ExternalInput").ap(),
        nc.dram_tensor("w_l", [H, H], F32, kind="ExternalInput").ap(),
        nc.dram_tensor("w_w", [H, H], F32, kind="ExternalInput").ap(),
        nc.dram_tensor("b_w", [H], F32, kind="ExternalInput").ap(),
        nc.dram_tensor("rpe_table", [H, NBKT], F32, kind="ExternalInput").ap(),
        nc.dram_tensor("rel_idx", [N, N], I32, kind="ExternalInput").ap(),
        nc.dram_tensor("out", [BLOC, N, C], F32, kind="ExternalOutput").ap(),
    )
    with tile.TileContext(nc) as tc, ExitStack() as ctx:
        _emit(ctx, tc, io)
    nc.compile()
    _CACHE["nc"] = nc
    return nc


def kernel(x, w_qkv, w_proj, b_proj, w_l, b_l, w_w, b_w, rpe_table, rel_idx,
           _trace=False):
    nc = _build()
    shared = {
        "w_qkv": np.ascontiguousarray(w_qkv, np.float32),
        "w_proj": np.ascontiguousarray(w_proj, np.float32),
        "b_proj": np.ascontiguousarray(b_proj, np.float32),
        "w_l": np.ascontiguousarray(w_l, np.float32),
        "w_w": np.ascontiguousarray(w_w, np.float32),
        "b_w": np.ascontiguousarray(b_w, np.float32),
        "rpe_table": np.ascontiguousarray(rpe_table, np.float32),
        "rel_idx": np.ascontiguousarray(rel_idx, np.int32),
    }
    x = np.ascontiguousarray(x, np.float32)
    in_maps = [dict(shared, x=x[i * BLOC:(i + 1) * BLOC]) for i in range(NCORES)]
    res = run_bass_kernel_spmd(nc, in_maps, core_ids=list(range(NCORES)),
                               trace=_trace)
    out = np.concatenate([res.results[i]["out"] for i in range(NCORES)], axis=0)
    if _trace:
        kernel.last_result = res
    return out


# revision 14
# speedup vs baseline: 2.2107x; 1.2635x over previous
"""Talking-heads attention Trainium2 kernel (Bass/Tile), 8-core data-parallel.

Problem: nn_Attention_talking_head — B=64, N=245, C=768, H=12, D=64,
RPE table (12, 1698) indexed by rel_idx (245, 245), talking-heads mixing
(12x12) before and after softmax, in/out projections.

Sharding: batch 64 -> 8 cores x 8 batches. Weights replicated. No collectives.

v2 design notes (vs v1):
  - bf16 attention pipeline (qT/kT/swt/pk/et/atw/v_sb + mixers); f32r GEMM
    anchors for QKV-from-x and the output projection.
  - max-subtraction dropped (|logit| <= ~2.5 for this problem's scales).
  - RPE bias add folded into the premix as a second accumulating matmul
    (identity lhsT), so softmax reads PSUM directly.
  - softmax normalization folded into the postmix by row-scaling the small
    w_w block-diagonal mixer with 1/rowsum (Pool engine).
  - per-tag PSUM pools (big/t1/mix/at = 2+2+2+2 banks) so consecutive j
    iterations pipeline instead of serializing on bank reuse.
  - paired evictions: QKV 2 e-chunks/bank, QK^T 2 heads/bank, T1 2 j/bank,
    postmix 2 j/bank -> fewer fixed-cost DVE/Act instructions.
  - atw uses j-major column order (col = j*8+nb); the AV eviction un-permutes
    back to n order via strided views.

b_l is mathematically a no-op (constant per softmax row) and is skipped.
"""
import os
import numpy as np
from contextlib import ExitStack

import concourse.bass as bass
import concourse.tile as tile
from concourse import bacc, mybir, library_config
from concourse.bass_utils import run_bass_kernel_spmd
from concourse.masks import make_identity

F32 = mybir.dt.float32
F32R = mybir.dt.float32r
BF16 = mybir.dt.bfloat16
I32 = mybir.dt.int32
I16 = mybir.dt.int16
AX = mybir.AxisListType.X
EXP = mybir.ActivationFunctionType.Exp
IDENT = mybir.ActivationFunctionType.Identity
ADD = mybir.AluOpType.add
MULT = mybir.AluOpType.mult

NCORES = 8
B, N, C, H, D = 64, 245, 768, 12, 64
BLOC = B // NCORES          # 8 batches per core
E = 3 * C                   # 2304
NBKT = 1698
SCALE = D ** -0.5
NPAD = 256
NGRP = 8                    # gather groups == packed nb slots
NJ = 31                     # packed tiles per batch; n = 31*nb + j, j in [0, NJ)
NJP = NJ * NGRP             # 248 packed col slots
NIDX = 7600                 # gather stream length per group (31*245 real + 5 pad)
CC = C // 128               # 6 contraction chunks
MCS = [(0, 128), (128, 117)]
P96 = H * NGRP              # 96 packed rows


def _emit(ctx: ExitStack, tc, io):
    nc = tc.nc
    x_d, wqkv_d, wproj_d, bproj_d, wl_d, ww_d, bw_d, rpe_d, rel_d, out_d = io

    const = ctx.enter_context(tc.tile_pool(name="const", bufs=1))
    ctx0 = ctx.enter_context(ExitStack())
    tmp = ctx0.enter_context(tc.tile_pool(name="tmp", bufs=1))
    ps_big = ctx.enter_context(tc.tile_pool(name="ps_big", bufs=2, space="PSUM"))
    ps_t1 = ctx.enter_context(tc.tile_pool(name="ps_t1", bufs=2, space="PSUM"))
    ps_mix = ctx.enter_context(tc.tile_pool(name="ps_mix", bufs=2, space="PSUM"))
    ps_at = ctx.enter_context(tc.tile_pool(name="ps_at", bufs=2, space="PSUM"))

    ident = const.tile([128, 128], F32)
    make_identity(nc, ident[:])
    ident_bf = const.tile([128, 128], BF16)
    nc.vector.tensor_copy(out=ident_bf[:], in_=ident[:])

    # ---- gather indices first (Act DGE queue; overlaps everything else) ----
    rel_flat = rel_d.rearrange("n m -> (n m)")
    idx32 = tmp.tile([128, NIDX // 16], I32, tag="idx32")
    nc.vector.memset(idx32[:], 0)
    for grp in range(NGRP):
        base = grp * NJ * N
        if grp < 7:
            nc.scalar.dma_start(
                out=idx32[grp * 16:(grp + 1) * 16, :],
                in_=rel_flat[base:base + NIDX].rearrange("(s p) -> p s", p=16))
        else:
            # group 7 has 28 real n rows (6860 idxs): 16x428 full + 12 tail
            nc.scalar.dma_start(
                out=idx32[grp * 16:(grp + 1) * 16, :428],
                in_=rel_flat[base:base + 6848].rearrange("(s p) -> p s", p=16))
            nc.scalar.dma_start(
                out=idx32[grp * 16:grp * 16 + 12, 428:429],
                in_=rel_flat[base + 6848:base + 6860].rearrange("(s p) -> p s", p=12))
    idx16 = tmp.tile([128, NIDX // 16], I16, tag="idx16")
    nc.vector.tensor_copy(out=idx16[:], in_=idx32[:])

    # ---- w_l / w_w transposes; block-diagonal mixers (bf16) ----
    wl_sb = tmp.tile([12, 12], F32, tag="wsml")
    nc.sync.dma_start(out=wl_sb[:], in_=wl_d[:, :])
    ps12 = ps_mix.tile([12, 12], F32, tag="mix")
    nc.tensor.transpose(out=ps12[:], in_=wl_sb[:], identity=ident[:12, :12])
    wlT_plain = const.tile([12, 12], F32)          # w_l^T (for RPE premix)
    nc.scalar.copy(out=wlT_plain[:], in_=ps12[:])
    wlT_scaled = tmp.tile([12, 12], F32, tag="wsml2")
    nc.scalar.mul(out=wlT_scaled[:], in_=ps12[:], mul=SCALE)

    ww_sb = tmp.tile([12, 12], F32, tag="wsml")
    nc.sync.dma_start(out=ww_sb[:], in_=ww_d[:, :])
    ps12b = ps_mix.tile([12, 12], F32, tag="mix")
    nc.tensor.transpose(out=ps12b[:], in_=ww_sb[:], identity=ident[:12, :12])
    wwT = tmp.tile([12, 12], F32, tag="wsml2")
    nc.scalar.copy(out=wwT[:], in_=ps12b[:])

    # nb-major packing: row p = nb*12 + h -> contiguous 12x12 diagonal blocks.
    bd1_f32 = tmp.tile([P96, P96], F32, tag="bd1f")
    nc.vector.memset(bd1_f32[:], 0.0)
    bd2_f32 = tmp.tile([P96, P96], F32, tag="bd2f")
    nc.vector.memset(bd2_f32[:], 0.0)
    for nb in range(NGRP):
        s = nb * H
        nc.gpsimd.dma_start(out=bd1_f32[s:s + H, s:s + H], in_=wlT_scaled[:])
        nc.gpsimd.dma_start(out=bd2_f32[s:s + H, s:s + H], in_=wwT[:])
    bd1 = const.tile([P96, P96], BF16)   # [(nb,h), (nb,g)] = SCALE*w_l[g,h]
    nc.vector.tensor_copy(out=bd1[:], in_=bd1_f32[:])
    bd2 = const.tile([P96, P96], BF16)   # [(nb,g), (nb,h)] = w_w[h,g]
    nc.vector.tensor_copy(out=bd2[:], in_=bd2_f32[:])

    # ---- premixed RPE table: mixed_rpe[g, k] = sum_h w_l[g,h] * rpe[h, k] ----
    rpe_sb = tmp.tile([12, NBKT], F32, tag="rpe")
    nc.sync.dma_start(out=rpe_sb[:], in_=rpe_d[:, :])
    mixed_rpe = tmp.tile([12, NBKT], F32, tag="rpemix")
    for o in range(0, NBKT, 512):
        w = min(512, NBKT - o)
        psr = ps_big.tile([12, 512], F32, tag="big")
        nc.tensor.matmul(out=psr[:, :w], lhsT=wlT_plain[:], rhs=rpe_sb[:, o:o + w],
                         start=True, stop=True)
        nc.scalar.copy(out=mixed_rpe[:, o:o + w], in_=psr[:, :w])

    # replicate across the 8 gather groups: table_rep[16*grp + c] = mixed_rpe[c]
    table_rep = tmp.tile([128, NBKT], F32, tag="trep")
    nc.vector.memset(table_rep[:], 0.0)
    for grp in range(NGRP):
        nc.scalar.dma_start(out=table_rep[grp * 16:grp * 16 + 12, :],
                            in_=mixed_rpe[:12, :])

    # ---- gather premixed bias, then repack to [(nb, h), j*245 + m] ----
    nc.gpsimd.load_library(library_config.ap_gather)
    bias_g = tmp.tile([128, NIDX], F32, tag="biasg")
    nc.gpsimd.ap_gather(
        out_ap=bias_g[:], in_ap=table_rep[:].unsqueeze(2), idxs_ap=idx16[:],
        channels=128, num_elems=NBKT, d=1, num_idxs=NIDX)
    nc.gpsimd.load_library(library_config.standard)

    packed_bias = const.tile([P96, NJ * N], BF16)
    for grp in range(NGRP):
        nc.gpsimd.dma_start(out=packed_bias[grp * H:(grp + 1) * H, :],
                            in_=bias_g[grp * 16:grp * 16 + 12, :NJ * N])

    # ---- small constants ----
    bw_exp = const.tile([128, CC, 1], F32)   # b_w[(t*128+p)//64]
    for t in range(CC):
        for half in range(2):
            h_idx = 2 * t + half
            nc.gpsimd.dma_start(
                out=bw_exp[half * 64:(half + 1) * 64, t, :],
                in_=bw_d[h_idx:h_idx + 1].unsqueeze(0).to_broadcast([64, 1]))
    bproj_sb = const.tile([128, C], F32)
    nc.gpsimd.dma_start(out=bproj_sb[:], in_=bproj_d[:].unsqueeze(0).to_broadcast([128, C]))
    ones_bf = const.tile([128, 1], BF16)
    nc.vector.memset(ones_bf[:], 1.0)
    zeros_c = const.tile([128, 1], F32)
    nc.vector.memset(zeros_c[:], 0.0)

    # ---- weight transposes (PE), paired evictions ----
    wqkvT = const.tile([128, CC, E], F32R)   # [c-part, c-chunk, e]
    for ec in range(E // 128):
        wt = tmp.tile([128, C], F32, tag="wload", bufs=3)
        nc.sync.dma_start(out=wt[:], in_=wqkv_d[ec * 128:(ec + 1) * 128, :])
        for cp in range(CC // 2):
            pst = ps_big.tile([128, 2, 128], F32, tag="big")
            for s in range(2):
                nc.tensor.transpose(out=pst[:, s, :],
                                    in_=wt[:, (2 * cp + s) * 128:(2 * cp + s + 1) * 128],
                                    identity=ident[:])
            if (ec + cp) % 2 == 0:
                nc.scalar.copy(out=wqkvT[:, 2 * cp:2 * cp + 2, ec * 128:(ec + 1) * 128],
                               in_=pst[:])
            else:
                nc.vector.tensor_copy(out=wqkvT[:, 2 * cp:2 * cp + 2, ec * 128:(ec + 1) * 128],
                                      in_=pst[:])

    wprojT = const.tile([128, CC, C], F32R)
    for ec in range(CC):
        wt = tmp.tile([128, C], F32, tag="wload", bufs=3)
        nc.sync.dma_start(out=wt[:], in_=wproj_d[ec * 128:(ec + 1) * 128, :])
        for cp in range(CC // 2):
            pst = ps_big.tile([128, 2, 128], F32, tag="big")
            for s in range(2):
                nc.tensor.transpose(out=pst[:, s, :],
                                    in_=wt[:, (2 * cp + s) * 128:(2 * cp + s + 1) * 128],
                                    identity=ident[:])
            if (ec + cp) % 2 == 0:
                nc.scalar.copy(out=wprojT[:, 2 * cp:2 * cp + 2, ec * 128:(ec + 1) * 128],
                               in_=pst[:])
            else:
                nc.vector.tensor_copy(out=wprojT[:, 2 * cp:2 * cp + 2, ec * 128:(ec + 1) * 128],
                                      in_=pst[:])

    ctx0.close()

    # ---- per-batch streaming pools ----
    xb_p = ctx.enter_context(tc.tile_pool(name="xb", bufs=2))
    xT_p = ctx.enter_context(tc.tile_pool(name="xT", bufs=2))
    qT_p = ctx.enter_context(tc.tile_pool(name="qT", bufs=2))
    kT_p = ctx.enter_context(tc.tile_pool(name="kT", bufs=2))
    v_p = ctx.enter_context(tc.tile_pool(name="v", bufs=2))
    swt_p = ctx.enter_context(tc.tile_pool(name="swt", bufs=2))
    pk_p = ctx.enter_context(tc.tile_pool(name="pk", bufs=3))
    et_p = ctx.enter_context(tc.tile_pool(name="et", bufs=4))
    bdj_p = ctx.enter_context(tc.tile_pool(name="bdj", bufs=4))
    at_p = ctx.enter_context(tc.tile_pool(name="at", bufs=2))
    oT_p = ctx.enter_context(tc.tile_pool(name="oT", bufs=2))
    y_p = ctx.enter_context(tc.tile_pool(name="y", bufs=2))
    st_p = ctx.enter_context(tc.tile_pool(name="st", bufs=4))

    STOP = os.environ.get("K_STOP", "full")
    for b in range(BLOC):
        # ---- load x_b and transpose to xT [c, n] (f32r, n padded to 256) ----
        xb = xb_p.tile([128, 2, C], F32)
        for mc, (mo, msz) in enumerate(MCS):
            nc.sync.dma_start(out=xb[:msz, mc, :], in_=x_d[b, mo:mo + msz, :])
        xT = xT_p.tile([128, CC, NPAD], F32R)
        nc.vector.tensor_copy(out=xT[:, :, N:],
                              in_=zeros_c[:, 0:1].to_broadcast([128, CC, NPAD - N]))
        for mc, (mo, msz) in enumerate(MCS):
            for cp in range(CC // 2):
                pst = ps_big.tile([128, 2, 128], F32, tag="big")
                for s in range(2):
                    nc.tensor.transpose(
                        out=pst[:, s, :msz],
                        in_=xb[:msz, mc, (2 * cp + s) * 128:(2 * cp + s + 1) * 128],
                        identity=ident[:msz, :msz])
                nc.scalar.copy(out=xT[:, 2 * cp:2 * cp + 2, mo:mo + msz],
                               in_=pst[:, :, :msz])

        # ---- QKV: q,k -> bf16 [hd, n]; v -> bf16 [m, hd] ----
        qT = qT_p.tile([128, CC, NPAD], BF16)
        kT = kT_p.tile([128, CC, NPAD], BF16)
        for ep in range(6):           # ec pairs: ep<3 -> q chunks, else k chunks
            psq = ps_big.tile([128, 2, NPAD], F32, tag="big")
            for s in range(2):
                ec = 2 * ep + s
                for cc in range(CC):
                    nc.tensor.matmul(out=psq[:, s, :],
                                     lhsT=wqkvT[:, cc, ec * 128:(ec + 1) * 128],
                                     rhs=xT[:, cc, :],
                                     start=(cc == 0), stop=(cc == CC - 1))
            dst = qT if ep < 3 else kT
            do = (2 * ep) % CC
            if ep % 2 == 0:
                nc.scalar.copy(out=dst[:, do:do + 2, :], in_=psq[:])
            else:
                nc.vector.tensor_copy(out=dst[:, do:do + 2, :], in_=psq[:])

        v_sb = v_p.tile([128, 2, C], BF16)        # [m, (h,d)]
        for mc, (mo, msz) in enumerate(MCS):
            for vc in range(2):
                psv = ps_big.tile([128, 384], F32, tag="big")
                for cc in range(CC):
                    nc.tensor.matmul(
                        out=psv[:msz], lhsT=xT[:, cc, mo:mo + msz],
                        rhs=wqkvT[:, cc, 2 * C + vc * 384:2 * C + (vc + 1) * 384],
                        start=(cc == 0), stop=(cc == CC - 1))
                nc.vector.tensor_copy(out=v_sb[:msz, mc, vc * 384:(vc + 1) * 384],
                                      in_=psv[:msz])

        if STOP == "qkv":
            continue
        # ---- b_w * colsum(v) ----
        SKIP_BWV = os.environ.get("K_SKIP_BWV")
        bwv = st_p.tile([128, CC, 1], F32, tag="bwv", bufs=2)
        if SKIP_BWV:
            nc.vector.memset(bwv[:], 0.0)
        for t in range(CC if not SKIP_BWV else 0):
            psvs = ps_at.tile([128, 1], F32, tag="at")
            for mc, (mo, msz) in enumerate(MCS):
                nc.tensor.matmul(out=psvs[:], lhsT=v_sb[:msz, mc, t * 128:(t + 1) * 128],
                                 rhs=ones_bf[:msz, :],
                                 start=(mc == 0), stop=(mc == 1))
            nc.vector.tensor_tensor(out=bwv[:, t, :], in0=psvs[:], in1=bw_exp[:, t, :], op=MULT)

        # ---- QK^T -> swt [m, (mc, j, nb, h)] bf16 ----
        # heads paired same-parity (h, h+2): lhsT partition offset must be
        # constant across matmuls that share a PSUM bank (HW constraint)
        swt = swt_p.tile([128, 2, NJ, NGRP, H], BF16)
        for mc, (mo, msz) in enumerate(MCS):
            for parity in range(2):
                for tp in range(3):
                    pss = ps_big.tile([128, 2, NPAD], F32, tag="big")
                    po = parity * 64
                    for s in range(2):
                        t = 2 * tp + s
                        nc.tensor.matmul(
                            out=pss[:msz, s, :],
                            lhsT=kT[po:po + 64, t, mo:mo + msz],
                            rhs=qT[po:po + 64, t, :],
                            start=True, stop=True)
                    src = pss[:msz, :, :NJP].rearrange("p h (nb j) -> p j nb h", j=NJ)
                    ha = 4 * tp + parity     # h = 2t + parity -> ha, ha+2
                    if (parity + tp) % 2 == 0:
                        nc.scalar.copy(out=swt[:msz, mc, :, :, ha:ha + 3:2], in_=src)
                    else:
                        nc.vector.tensor_copy(out=swt[:msz, mc, :, :, ha:ha + 3:2], in_=src)

        if STOP == "qk":
            continue
        # ---- per-j packed attention (j processed in pairs) ----
        atw = at_p.tile([128, 2, H, NJP], BF16)   # cols j-major: col = j*8 + nb
        for jp in range((NJ + 1) // 2):
            js = [2 * jp] + ([2 * jp + 1] if 2 * jp + 1 < NJ else [])
            # T1: packed S^T [(nb,h), m] for both j of the pair -> one bank
            pspk = ps_t1.tile([P96, 2, NPAD], BF16, tag="t1")
            for ji, j in enumerate(js):
                for mc, (mo, msz) in enumerate(MCS):
                    nc.tensor.transpose(
                        out=pspk[:, ji, mo:mo + msz],
                        in_=swt[:msz, mc, j, :, :].rearrange("p nb h -> p (nb h)"),
                        identity=ident_bf[:msz, :msz])
            pk = pk_p.tile([P96, 2, NPAD], BF16, tag="pk")
            nc.vector.tensor_copy(out=pk[:, :len(js), :N], in_=pspk[:, :len(js), :N])

            ssum = st_p.tile([P96, 2], F32, tag="ss")
            ets = []
            for ji, j in enumerate(js):
                # premix (block-diag) + RPE bias, both as matmuls into PSUM
                psm = ps_mix.tile([P96, NPAD], F32, tag="mix")
                nc.tensor.matmul(out=psm[:, :N], lhsT=bd1[:], rhs=pk[:, ji, :N],
                                 start=True, stop=False)
                nc.tensor.matmul(out=psm[:, :N], lhsT=ident_bf[:P96, :P96],
                                 rhs=packed_bias[:, j * N:(j + 1) * N],
                                 start=False, stop=True)
                et = et_p.tile([P96, NPAD], BF16, tag="et")
                # zero the pad cols so the mc1 postmix can emit full 128 rows
                nc.gpsimd.memset(et[:, N:], 0.0)
                nc.scalar.activation(out=et[:, :N], in_=psm[:, :N], func=EXP,
                                     scale=1.0, accum_out=ssum[:, ji:ji + 1])
                ets.append(et)
            rec = st_p.tile([P96, 2], F32, tag="rc")
            nc.vector.reciprocal(out=rec[:, :len(js)], in_=ssum[:, :len(js)])

            psat = ps_at.tile([128, 2, 2, P96], F32, tag="at")  # [m, (jj, mc), 96]
            for ji, j in enumerate(js):
                bd2j = bdj_p.tile([P96, P96], BF16, tag="bdj")
                nc.vector.tensor_scalar_mul(bd2j[:], bd2[:], rec[:, ji:ji + 1])
                for mc in range(2):
                    # mc1 uses a full 128-col lhsT slice (real m + zeroed pad)
                    nc.tensor.matmul(out=psat[:, ji, mc, :],
                                     lhsT=ets[ji][:, mc * 128:mc * 128 + 128],
                                     rhs=bd2j[:],
                                     start=True, stop=True)
            for ji, j in enumerate(js):
                src = psat[:, ji, :, :].rearrange("p m (n h) -> p m h n", n=NGRP)
                if jp % 2 == 0:
                    nc.vector.tensor_copy(
                        out=atw[:, :, :, j * NGRP:(j + 1) * NGRP], in_=src)
                else:
                    nc.scalar.copy(
                        out=atw[:, :, :, j * NGRP:(j + 1) * NGRP], in_=src)

        if STOP == "jloop":
            continue
        # ---- AV (+ b_w colsum term) -> outT [(h,d), n] f32r (n order) ----
        outT = oT_p.tile([128, CC, NJP], F32R)
        for half in range(2):
            for tp in range(3):
                psav = ps_mix.tile([64, 2, NPAD], F32, tag="mix")
                for s in range(2):
                    h = 4 * tp + 2 * s + half
                    for mc, (mo, msz) in enumerate(MCS):
                        nc.tensor.matmul(out=psav[:, s, :NJP],
                                         lhsT=v_sb[:msz, mc, h * 64:(h + 1) * 64],
                                         rhs=atw[:msz, mc, h, :],
                                         start=(mc == 0), stop=(mc == 1))
                for s in range(2):
                    t = 2 * tp + s
                    nc.scalar.activation(
                        out=outT[half * 64:(half + 1) * 64, t, :].rearrange(
                            "p (nb j) -> p nb j", j=NJ),
                        in_=psav[:64, s, :NJP].rearrange("p (j nb) -> p nb j", nb=NGRP),
                        func=IDENT, bias=bwv[half * 64:(half + 1) * 64, t, :], scale=1.0)

        if STOP == "av":
            continue
        # ---- projection + b_proj -> y -> DRAM ----
        for mc, (mo, msz) in enumerate(MCS):
            y = y_p.tile([128, C], F32)
            for half in range(2):
                psy = ps_big.tile([128, 384], F32, tag="big")
                for cc in range(CC):
                    nc.tensor.matmul(
                        out=psy[:msz], lhsT=outT[:, cc, mo:mo + msz],
                        rhs=wprojT[:, cc, half * 384:(half + 1) * 384],
                        start=(cc == 0), stop=(cc == CC - 1))
                nc.vector.tensor_tensor(out=y[:msz, half * 384:(half + 1) * 384],
                                        in0=psy[:msz],
                                        in1=bproj_sb[:msz, half * 384:(half + 1) * 384],
                                        op=ADD)
            nc.sync.dma_start(out=out_d[b, mo:mo + msz, :], in_=y[:msz, :])


_CACHE = {}


def _build():
    if "nc" in _CACHE:
        return _CACHE["nc"]
    nc = bacc.Bacc("TRN2", target_bir_lowering=False, debug=False, num_devices=NCORES)
    io = (
        nc.dram_tensor("x", [BLOC, N, C], F32, kind="ExternalInput").ap(),
        nc.dram_tensor("w_qkv", [E, C], F32, kind="ExternalInput").ap(),
        nc.dram_tensor("w_proj", [C, C], F32, kind="ExternalInput").ap(),
        nc.dram_tensor("b_proj", [C], F32, kind="ExternalInput").ap(),
        nc.dram_tensor("w_l", [H, H], F32, kind="ExternalInput").ap(),
        nc.dram_tensor("w_w", [H, H], F32, kind="ExternalInput").ap(),
        nc.dram_tensor("b_w", [H], F32, kind="ExternalInput").ap(),
        nc.dram_tensor("rpe_table", [H, NBKT], F32, kind="ExternalInput").ap(),
        nc.dram_tensor("rel_idx", [N, N], I32, kind="ExternalInput").ap(),
        nc.dram_tensor("out", [BLOC, N, C], F32, kind="ExternalOutput").ap(),
    )
    with tile.TileContext(nc) as tc, ExitStack() as ctx:
        _emit(ctx, tc, io)
    nc.compile()
    _CACHE["nc"] = nc
    return nc


def kernel(x, w_qkv, w_proj, b_proj, w_l, b_l, w_w, b_w, rpe_table, rel_idx,
           _trace=False):
    nc = _build()
    shared = {
        "w_qkv": np.ascontiguousarray(w_qkv, np.float32),
        "w_proj": np.ascontiguousarray(w_proj, np.float32),
        "b_proj": np.ascontiguousarray(b_proj, np.float32),
        "w_l": np.ascontiguousarray(w_l, np.float32),
        "w_w": np.ascontiguousarray(w_w, np.float32),
        "b_w": np.ascontiguousarray(b_w, np.float32),
        "rpe_table": np.ascontiguousarray(rpe_table, np.float32),
        "rel_idx": np.ascontiguousarray(rel_idx, np.int32),
    }
    x = np.ascontiguousarray(x, np.float32)
    in_maps = [dict(shared, x=x[i * BLOC:(i + 1) * BLOC]) for i in range(NCORES)]
    res = run_bass_kernel_spmd(nc, in_maps, core_ids=list(range(NCORES)),
                               trace=_trace)
    out = np.concatenate([res.results[i]["out"] for i in range(NCORES)], axis=0)
    if _trace:
        kernel.last_result = res
    return out
